# revision 32
# baseline (speedup 1.0000x reference)
"""Two-layer GAT on 8 Trainium2 NeuronCores (Bass/Tile).

Strategy (graph/data parallel, dst-sharded):
- Host: add self-loops, sort edges by dst, shard dst-node ranges across 8
  cores, greedily pack each core's edges into 128-edge tiles grouped into
  128-node blocks (8 tiles/block); structure tables (wrapped gather idx,
  one-hot S01 [e,d] / S10 [d,e] per tile) precomputed on host.
- Device phase A (redundant on every core): HT[n] = [h1(n) | e_src1(n)]
  (fp16) for all 20000 nodes via x @ [W1|As] (TensorE), written to HBM.
  e_dst1 per packed node kept in SBUF (A2, interleaved). Layer-2 gather
  descriptors for the first blocks are pre-generated on idle GpSimd
  (SWDGE prepare_only on queues 1-3), triggered after the AllGather.
- Layer-1 edge phase, per 128-node block: one 1024-row SWDGE gather of
  HT[src]; e_dst broadcast dst->edges via S10 matmuls into PSUM;
  e = lrelu(es+ed); ex = exp(e - ln256) (shift cancels in softmax);
  messages formed in place (g1 *= ex per head); scatter-sum + ex-sum via
  S01^T matmuls into block PSUM; rinv = 1/sum ex; y = relu(o1)+exp(min(o1,0))
  (elu + 1, fp16); y^T via DMA-xbar transposes; layer-2 linear
  h2e = y^T @ W2e - colsum(W2e) (folds the elu "-1") inline; h2/es2 to
  H2TL, ed2 kept in SBUF.
- Halo exchange: AllGather of H2TL (h2 | es2 table) across 8 cores.
- Layer-2 edge phase: same machinery with H=1, C=64; gathers for blocks
  0..8 fire from pre-generated descriptors, the rest generate inline.
- Host: inverse-permute the 8 output shards into the full [20000, 64].
"""
import os
import sys
import numpy as np

sys.path.insert(0, '/opt/trn_rl_repo')

import concourse.bacc as bacc
import concourse.bass as bass
import concourse.mybir as mybir
import concourse.tile as tile
from concourse.masks import make_identity

F16 = mybir.dt.float16
F32 = mybir.dt.float32
I16 = mybir.dt.int16

N_NODES = 20000
IN_F = 128
HID = 1024          # 32 heads x 32 ch
H1, C1 = 32, 32
OUT_EMB = 64
NC = 8
SHARD = N_NODES // NC
K_TILES = 8
TILE_E = 128
NEG_SLOPE = 0.2
LN_SHIFT = float(np.log(256.0))
HT_ROW = 1152       # 1024 h + 32 es + 96 pad (2304B = 9*256)
EPS1 = float(1e-16 / 256.0)
PREP_DEPTH = 9      # layer-2 gathers pre-generated (3 SWDGE queues x 3)

# ---------------------------------------------------------------- host planning


def build_plan(edge_index: np.ndarray):
    ei = np.asarray(edge_index)
    loops = np.arange(N_NODES, dtype=ei.dtype)
    src = np.concatenate([ei[0], loops])
    dst = np.concatenate([ei[1], loops])
    order = np.argsort(dst, kind='stable')
    src_s = src[order].astype(np.int64)
    dst_s = dst[order].astype(np.int64)

    per_core = []
    max_nb = 0
    for c in range(NC):
        lo, hi = c * SHARD, (c + 1) * SHARD
        m = (dst_s >= lo) & (dst_s < hi)
        csrc, cdst = src_s[m], dst_s[m]
        nodes, starts, counts = np.unique(cdst, return_index=True, return_counts=True)
        blocks = []
        bi_nodes, bi_tiles = [], []
        t_src, t_seg = [], []

        def close_tile():
            nonlocal t_src, t_seg
            if t_src:
                bi_tiles.append((t_src, t_seg))
                t_src, t_seg = [], []

        def close_block():
            nonlocal bi_nodes, bi_tiles
            close_tile()
            if bi_nodes:
                blocks.append((bi_nodes, bi_tiles))
                bi_nodes, bi_tiles = [], []

        for n, st, cnt in zip(nodes, starts, counts):
            if len(t_src) + cnt > TILE_E:
                close_tile()
            need_new_tile = not t_src
            if len(bi_nodes) >= 128 or (need_new_tile and len(bi_tiles) >= K_TILES):
                close_block()
            local = len(bi_nodes)
            bi_nodes.append(int(n))
            t_src.extend(csrc[st:st + cnt].tolist())
            t_seg.extend([local] * int(cnt))
        close_block()
        per_core.append(blocks)
        max_nb = max(max_nb, len(blocks))

    NB = max_nb
    T = NB * K_TILES
    NPAD = NB * 128
    plan = {
        'NB': NB, 'T': T, 'NPAD': NPAD,
        'src_tiles': np.zeros((NC, T, TILE_E), np.int64),
        'seg_tiles': np.full((NC, T, TILE_E), -1.0, np.float32),
        'node_order': np.full((NC, NPAD), -1, np.int64),
    }
    for c, blocks in enumerate(per_core):
        for b, (bnodes, btiles) in enumerate(blocks):
            for r, n in enumerate(bnodes):
                plan['node_order'][c, b * 128 + r] = n
            for k, (tsrc, tseg) in enumerate(btiles):
                t = b * K_TILES + k
                plan['src_tiles'][c, t, :len(tsrc)] = tsrc
                plan['seg_tiles'][c, t, :len(tseg)] = tseg
    node_to_row = np.zeros(N_NODES, np.int64)
    for c in range(NC):
        valid = plan['node_order'][c] >= 0
        node_to_row[plan['node_order'][c][valid]] = c * NPAD + np.nonzero(valid)[0]
    plan['node_to_row'] = node_to_row
    return plan


def wrap_idx(idx_tiles: np.ndarray, NB: int) -> np.ndarray:
    """[T,128] -> dma_gather wrapped layout [128, NB*64] int16 (batch = 8 tiles)."""
    out = np.zeros((128, NB * 64), np.int16)
    flat = idx_tiles.reshape(NB, K_TILES * TILE_E)
    for b in range(NB):
        w = np.zeros((16, 64), np.int16)
        v = flat[b]
        idx = np.arange(1024)
        w[idx % 16, idx // 16] = v.astype(np.int16)
        out[:, b * 64:(b + 1) * 64] = np.tile(w, (8, 1))
    return out


# ---------------------------------------------------------------- device program

def build_program(NB, b1_nonzero, b2_nonzero, use_collective=True):
    T = NB * K_TILES
    NPAD = NB * 128
    use_mq = bool(int(os.environ.get("GAT_MQ", "1")))
    # prepare_only descriptor pre-generation deadlocks on hardware (works in
    # the scheduler sim); keep the path behind a flag, default off
    use_prep = use_mq and bool(int(os.environ.get("GAT_PREP", "0")))
    use_dmatp = bool(int(os.environ.get("GAT_DMATP", "1")))
    n_prep = min(PREP_DEPTH, NB) if use_prep else 0

    nc = bacc.Bacc("TRN2", target_bir_lowering=False, debug=False, num_devices=NC,
                   num_swdge_queues=4 if use_mq else 1,
                   dynamic_dma_scratch_size=49152 if use_prep else 32768)

    def din(name, shape, dt):
        return nc.dram_tensor(name, shape, dt, kind="ExternalInput")

    XT_G = din("XT_G", [128, N_NODES], F16)
    XTP = din("XTP", [128, NPAD], F16)
    W1P = din("W1P", [128, 1056], F16)
    WDP = din("WDP", [128, 32], F16)
    W2P = din("W2P", [128, 8 * 66], F16)
    C2T = din("C2T", [128, 66], F32)
    IDX1 = din("IDX1", [128, NB * 64], I16)
    IDX2 = din("IDX2", [128, NB * 64], I16)
    S01T = din("S01T", [128, T * 128], F16)
    S10T = din("S10T", [128, T * 128], F16)
    if b1_nonzero:
        B1 = din("B1", [128, HID], F32)
    if b2_nonzero:
        B2 = din("B2", [128, OUT_EMB], F32)

    OUT = nc.dram_tensor("OUT", [NPAD, OUT_EMB], F32, kind="ExternalOutput")

    HT = nc.dram_tensor("HT", [N_NODES, HT_ROW], F16)
    H2TL = nc.dram_tensor("H2TL", [NPAD, 128], F16)
    H2TF = nc.dram_tensor("H2TF", [NC * NPAD, 128], F16, addr_space="Shared")

    with tile.TileContext(nc) as tc:
        with (
            tc.tile_pool(name="const", bufs=1) as cpool,
            tc.tile_pool(name="sb", bufs=2) as sb,
            tc.tile_pool(name="psum", bufs=2, space="PSUM") as pp,
        ):
            # ---- resident tables
            xtp = cpool.tile([128, NPAD], F16, tag="xtp")
            nc.sync.dma_start(xtp[:], XTP[:, :])
            w1p = cpool.tile([128, 1056], F16, tag="w1p")
            nc.sync.dma_start(w1p[:], W1P[:, :])
            wdp = cpool.tile([128, 32], F16, tag="wdp")
            nc.sync.dma_start(wdp[:], WDP[:, :])
            w2p = cpool.tile([128, 8 * 66], F16, tag="w2p")
            nc.sync.dma_start(w2p[:], W2P[:, :])
            c2t = cpool.tile([128, 66], F32, tag="c2t")
            nc.sync.dma_start(c2t[:], C2T[:, :])
            idx1 = cpool.tile([128, NB * 64], I16, tag="idx1")
            nc.sync.dma_start(idx1[:], IDX1[:, :])
            idx2 = cpool.tile([128, NB * 64], I16, tag="idx2")
            nc.sync.dma_start(idx2[:], IDX2[:, :])
            if b1_nonzero:
                b1t = cpool.tile([128, HID], F32, tag="b1t")
                nc.sync.dma_start(b1t[:], B1[:, :])
            if b2_nonzero:
                b2t = cpool.tile([128, OUT_EMB], F32, tag="b2t")
                nc.sync.dma_start(b2t[:], B2[:, :])
            ed1 = cpool.tile([128, NB * 32], F16, tag="ed1")
            ed2sb = cpool.tile([128, NB], F16, tag="ed2sb")
            negshift = cpool.tile([128, 1], F32, tag="negshift")
            nc.gpsimd.memset(negshift[:], -LN_SHIFT)
            ident = cpool.tile([128, 128], F16, tag="ident")
            make_identity(nc, ident[:])

            g2sems = [nc.alloc_semaphore(f"g2dma{q}") for q in range(3)]
            g2_tiles = {}

            g2_bufs = PREP_DEPTH + 3 if use_prep else 4

            def emit_g2_gather(j, prepare):
                g2t = sb.tile([128, K_TILES, 128], F16, tag="g2",
                              bufs=g2_bufs, name=f"g2_{j}")
                g2_tiles[j] = g2t
                if prepare:
                    nc.gpsimd.dma_gather(
                        out_ap=g2t[:], in_ap=H2TF[:, :],
                        idxs_ap=idx2[:, j * 64:(j + 1) * 64],
                        num_idxs=1024, num_idxs_reg=1024, elem_size=128,
                        queue_num=1 + j % 3, prepare_only=True, sem=g2sems[j % 3])
                    return
                for hf in range(2):
                    nc.gpsimd.dma_gather(
                        out_ap=g2t[:, hf * 4:(hf + 1) * 4, :], in_ap=H2TF[:, :],
                        idxs_ap=idx2[:, j * 64 + hf * 32:j * 64 + (hf + 1) * 32],
                        num_idxs=512, num_idxs_reg=512, elem_size=128,
                        queue_num=(2 * j + hf) % 4 if use_mq else 0)

            # ================= phase A: HT = [h | es] for all nodes =========
            # (layer-2 gather preps + per-block e_dst1 (A2) interleaved)
            n_a_tiles = (N_NODES + 127) // 128
            CHUNK = 20 * 128
            prep_at = {40 + 13 * j: j for j in range(n_prep)}
            xa = None
            for m in range(n_a_tiles):
                n0 = m * 128
                nn = min(128, N_NODES - n0)
                if m % 20 == 0:
                    c0 = m * 128
                    cw = min(CHUNK, N_NODES - c0)
                    xa = sb.tile([128, CHUNK], F16, tag="xa")
                    nc.sync.dma_start(xa[:, 0:cw], XT_G[:, c0:c0 + cw])
                pa_h = pp.tile([128, 1024], F32, tag="ob")
                pa_es = pp.tile([128, 512], F32, tag="blk")
                lhsT = xa[:, n0 - (m // 20) * CHUNK:n0 - (m // 20) * CHUNK + nn]
                stage = sb.tile([128, HT_ROW], F16, tag="aStage", bufs=3)
                nc.tensor.matmul(pa_h[0:nn, 0:512], lhsT, w1p[:, 0:512], start=True, stop=True)
                nc.scalar.copy(stage[0:nn, 0:512], pa_h[0:nn, 0:512])
                nc.tensor.matmul(pa_h[0:nn, 512:1024], lhsT, w1p[:, 512:1024], start=True, stop=True)
                nc.tensor.matmul(pa_es[0:nn, 0:32], lhsT, w1p[:, 1024:1056], start=True, stop=True)
                nc.vector.tensor_copy(stage[0:nn, 512:1024], pa_h[0:nn, 512:1024])
                nc.vector.tensor_copy(stage[0:nn, 1024:1056], pa_es[0:nn, 0:32])
                nc.sync.dma_start(HT[n0:n0 + nn, :], stage[0:nn, :])
                # A2: e_dst1 per packed block, kept in SBUF
                if m % 7 == 0 and m // 7 < NB:
                    b = m // 7
                    ped = pp.tile([128, 512], F32, tag="blk")
                    nc.tensor.matmul(ped[:, 0:32], xtp[:, b * 128:(b + 1) * 128], wdp[:],
                                     start=True, stop=True)
                    nc.vector.tensor_copy(ed1[:, b * 32:(b + 1) * 32], ped[:, 0:32])
                if m in prep_at:
                    emit_g2_gather(prep_at[m], prepare=True)

            # ================= layer-1 edge phase (+ inline layer-2 linear) ==
            for b in range(NB):
                # two 512-row gathers: a full-window 1024-row gather serializes
                # desc-gen behind its own drain on the SWDGE ring
                g1a = sb.tile([128, 4, HT_ROW], F16, tag="g1", name="g1a")
                nc.gpsimd.dma_gather(
                    out_ap=g1a[:], in_ap=HT[:, :],
                    idxs_ap=idx1[:, b * 64:b * 64 + 32],
                    num_idxs=512, num_idxs_reg=512, elem_size=HT_ROW,
                    queue_num=(2 * b) % 4 if (use_mq and not use_prep) else 0)
                g1b = sb.tile([128, 4, HT_ROW], F16, tag="g1", name="g1b")
                nc.gpsimd.dma_gather(
                    out_ap=g1b[:], in_ap=HT[:, :],
                    idxs_ap=idx1[:, b * 64 + 32:(b + 1) * 64],
                    num_idxs=512, num_idxs_reg=512, elem_size=HT_ROW,
                    queue_num=(2 * b + 1) % 4 if (use_mq and not use_prep) else 0)
                g1h = [g1a, g1b]
                s01 = sb.tile([128, K_TILES, 128], F16, tag="s01")
                nc.sync.dma_start(s01[:], S01T[:, b * 1024:(b + 1) * 1024])
                s10 = sb.tile([128, K_TILES, 128], F16, tag="s10")
                nc.sync.dma_start(s10[:], S10T[:, b * 1024:(b + 1) * 1024])

                # block psum: [0:256] ed_e | [256:288] sbp | [288:354] ph
                blk = pp.tile([128, 512], F32, tag="blk")
                ed_e = blk[:, 0:256]
                for k in range(K_TILES):
                    nc.tensor.matmul(ed_e[:, k * 32:(k + 1) * 32], s10[:, k, :],
                                     ed1[:, b * 32:(b + 1) * 32],
                                     start=True, stop=True)
                # e chain (fp16)
                e_f = sb.tile([128, K_TILES * 32], F16, tag="e_f")
                for hf in range(2):
                    nc.vector.tensor_tensor(
                        out=e_f[:, hf * 128:(hf + 1) * 128]
                             .rearrange("p (k h) -> p k h", k=4),
                        in0=g1h[hf][:, :, 1024:1056],
                        in1=ed_e[:, hf * 128:(hf + 1) * 128]
                             .rearrange("p (k h) -> p k h", k=4),
                        op=mybir.AluOpType.add)
                lr = sb.tile([128, K_TILES * 32], F16, tag="lr")
                nc.vector.scalar_tensor_tensor(
                    out=lr[:], in0=e_f[:], scalar=NEG_SLOPE, in1=e_f[:],
                    op0=mybir.AluOpType.mult, op1=mybir.AluOpType.max)
                ex = sb.tile([128, K_TILES * 32], F16, tag="ex")
                nc.scalar.activation(ex[:], lr[:], mybir.ActivationFunctionType.Exp,
                                     bias=negshift[:, 0:1])
                # messages (channel-major h: inner dim head, stride-1 on both
                # DVE operands); two halves so scatter matmuls overlap
                ob = pp.tile([128, HID], F32, tag="ob")
                sbp = blk[:, 256:288]
                for hf in range(2):
                    mh = sb.tile([128, 4, HID], F16, tag="msg", name=f"msg{hf}")
                    nc.vector.tensor_tensor(
                        out=mh[:].rearrange("p k (c h) -> p k c h", c=C1),
                        in0=g1h[hf][:, :, 0:1024].rearrange("p k (c h) -> p k c h", c=C1),
                        in1=ex[:, hf * 128:(hf + 1) * 128]
                             .rearrange("p (k h) -> p k h", k=4)
                             .unsqueeze(2).to_broadcast([128, 4, C1, H1]),
                        op=mybir.AluOpType.mult)
                    for k in range(hf * 4, hf * 4 + 4):
                        lhsT = s01[:, k, :]
                        nc.tensor.matmul(sbp[:, 0:32], lhsT, ex[:, k * 32:(k + 1) * 32],
                                         start=(k == 0), stop=(k == K_TILES - 1))
                        nc.tensor.matmul(ob[:, 0:512], lhsT, mh[:, k % 4, 0:512],
                                         start=(k == 0), stop=(k == K_TILES - 1))
                        nc.tensor.matmul(ob[:, 512:1024], lhsT, mh[:, k % 4, 512:1024],
                                         start=(k == 0), stop=(k == K_TILES - 1))

                # ---- block finishing (fp16 elu path; y = elu + 1)
                sp = sb.tile([128, 32], F32, tag="sp")
                nc.vector.tensor_scalar_add(sp[:], sbp[:, 0:32], EPS1)
                rinv = sb.tile([128, 32], F32, tag="rinv")
                nc.vector.reciprocal(rinv[:], sp[:])
                o1h = sb.tile([128, HID], F16, tag="o1h")
                nc.vector.tensor_tensor(
                    out=o1h[:].rearrange("p (c h) -> p c h", c=C1),
                    in0=ob[:].rearrange("p (c h) -> p c h", c=C1),
                    in1=rinv[:].unsqueeze(1).to_broadcast([128, C1, H1]),
                    op=mybir.AluOpType.mult)
                if b1_nonzero:
                    nc.vector.tensor_tensor(out=o1h[:], in0=o1h[:], in1=b1t[:, :],
                                            op=mybir.AluOpType.add)
                tmin = sb.tile([128, HID], F16, tag="tmin")
                nc.vector.tensor_scalar_min(tmin[:], o1h[:], 0.0)
                nc.scalar.activation(tmin[:], tmin[:], mybir.ActivationFunctionType.Exp)
                yb = sb.tile([128, HID], F16, tag="yb")
                nc.vector.scalar_tensor_tensor(
                    out=yb[:], in0=o1h[:], scalar=0.0, in1=tmin[:],
                    op0=mybir.AluOpType.max, op1=mybir.AluOpType.add)
                # y^T via PE transposes into one f16 psum bank
                tpb = pp.tile([128, HID], F16, tag="tp", bufs=2)
                for kk in range(8):
                    nc.tensor.transpose(tpb[:, kk * 128:(kk + 1) * 128],
                                        yb[:, kk * 128:(kk + 1) * 128], ident[:])
                tstage = sb.tile([128, HID], F16, tag="tst")
                nc.scalar.copy(tstage[:], tpb[:])
                # layer-2 linear: h2e = y^T @ W2e - colsum(W2e)  (elu -1 folded)
                ph = blk[:, 288:354]
                for k in range(8):
                    nc.tensor.matmul(ph[:, 0:66], tstage[:, k * 128:(k + 1) * 128],
                                     w2p[:, k * 66:(k + 1) * 66],
                                     start=(k == 0), stop=(k == 7))
                h2s = sb.tile([128, 128], F16, tag="h2s")
                nc.vector.tensor_tensor(out=h2s[:, 0:65], in0=ph[:, 0:65],
                                        in1=c2t[:, 0:65], op=mybir.AluOpType.subtract)
                nc.vector.tensor_tensor(out=ed2sb[:, b:b + 1], in0=ph[:, 65:66],
                                        in1=c2t[:, 65:66], op=mybir.AluOpType.subtract)
                nc.sync.dma_start(H2TL[b * 128:(b + 1) * 128, :], h2s[:])

            # ================= halo exchange =================
            if use_collective:
                nc.gpsimd.collective_compute(
                    "AllGather",
                    mybir.AluOpType.bypass,
                    ins=[H2TL.ap().opt()],
                    outs=[H2TF.ap().opt()],
                    replica_groups=[list(range(NC))],
                )
            else:
                nc.sync.dma_start(H2TF[0:NPAD, :], H2TL[:, :])

            # ================= layer-2 edge phase =================
            for j in range(n_prep):
                nc.gpsimd.trigger_dma(count=1, queue_num=1 + j % 3)
            for b in range(NB):
                if b + n_prep < NB:
                    emit_g2_gather(b + n_prep, prepare=False)
                g2 = g2_tiles[b]
                s01b = sb.tile([128, K_TILES, 128], F16, tag="s01L2", bufs=3)
                nc.sync.dma_start(s01b[:], S01T[:, b * 1024:(b + 1) * 1024])
                s10b = sb.tile([128, K_TILES, 128], F16, tag="s10L2", bufs=3)
                nc.sync.dma_start(s10b[:], S10T[:, b * 1024:(b + 1) * 1024])

                blk2 = pp.tile([128, 512], F32, tag="blk")
                ed2_e = blk2[:, 0:256]
                for k in range(K_TILES):
                    nc.tensor.matmul(ed2_e[:, k:k + 1], s10b[:, k, :],
                                     ed2sb[:, b:b + 1], start=True, stop=True)
                e2 = sb.tile([128, K_TILES], F16, tag="e2", bufs=3)
                nc.vector.tensor_tensor(
                    out=e2[:].unsqueeze(2),
                    in0=g2[:, :, 64:65],
                    in1=ed2_e[:, 0:K_TILES].unsqueeze(2),
                    op=mybir.AluOpType.add)
                nc.vector.scalar_tensor_tensor(
                    out=e2[:], in0=e2[:], scalar=NEG_SLOPE, in1=e2[:],
                    op0=mybir.AluOpType.mult, op1=mybir.AluOpType.max)
                ex2 = sb.tile([128, K_TILES], F16, tag="ex2", bufs=3)
                nc.scalar.activation(ex2[:], e2[:], mybir.ActivationFunctionType.Exp,
                                     bias=negshift[:, 0:1])
                msg2 = sb.tile([128, K_TILES, OUT_EMB], F16, tag="msg2", bufs=3)
                nc.vector.tensor_tensor(
                    out=msg2[:],
                    in0=g2[:, :, 0:64],
                    in1=ex2[:].unsqueeze(2).to_broadcast([128, K_TILES, OUT_EMB]),
                    op=mybir.AluOpType.mult)
                ob2 = pp.tile([128, HID], F32, tag="ob")
                sb2 = blk2[:, 256:288]
                for k in range(K_TILES):
                    lhsT = s01b[:, k, :]
                    nc.tensor.matmul(sb2[:, 0:1], lhsT, ex2[:, k:k + 1],
                                     start=(k == 0), stop=(k == K_TILES - 1))
                    nc.tensor.matmul(ob2[:, 0:64], lhsT, msg2[:, k, :],
                                     start=(k == 0), stop=(k == K_TILES - 1))
                sp2 = sb.tile([128, 1], F32, tag="sp2", bufs=3)
                nc.vector.tensor_scalar_add(sp2[:], sb2[:, 0:1], EPS1)
                rinv2 = sb.tile([128, 1], F32, tag="rinv2", bufs=3)
                nc.vector.reciprocal(rinv2[:], sp2[:])
                o2 = sb.tile([128, OUT_EMB], F32, tag="o2", bufs=3)
                nc.vector.tensor_scalar_mul(o2[:], ob2[:, 0:64], rinv2[:, 0:1])
                if b2_nonzero:
                    nc.vector.tensor_tensor(out=o2[:], in0=o2[:], in1=b2t[:, :],
                                            op=mybir.AluOpType.add)
                nc.sync.dma_start(OUT[b * 128:(b + 1) * 128, :], o2[:])

    nc.compile()
    return nc


# ---------------------------------------------------------------- driver

_CACHE = {}


def _get_program(NB, b1_nonzero, b2_nonzero):
    key = (NB, b1_nonzero, b2_nonzero)
    if key not in _CACHE:
        _CACHE[key] = build_program(NB, b1_nonzero, b2_nonzero)
    return _CACHE[key]


def kernel(x, edge_index, W1, att_src1, att_dst1, b1, W2, att_src2, att_dst2, b2,
           _return_results=False):
    x = np.asarray(x); edge_index = np.asarray(edge_index)
    W1 = np.asarray(W1); att_src1 = np.asarray(att_src1); att_dst1 = np.asarray(att_dst1)
    b1 = np.asarray(b1); W2 = np.asarray(W2)
    att_src2 = np.asarray(att_src2); att_dst2 = np.asarray(att_dst2); b2 = np.asarray(b2)

    plan = build_plan(edge_index)
    NB, T, NPAD = plan['NB'], plan['T'], plan['NPAD']

    # fused weights (host, fp32 math then fp16); hidden in channel-major
    # layout (col c*32+h) so the per-head DVE broadcasts keep stride-1 inner
    W1r = W1.reshape(IN_F, H1, C1)
    As = np.einsum('fhc,hc->fh', W1r, att_src1)
    Ad = np.einsum('fhc,hc->fh', W1r, att_dst1)
    W1cm = W1r.transpose(0, 2, 1).reshape(IN_F, HID)
    W1P = np.concatenate([W1cm, As], axis=1).astype(np.float16)           # [128, 1056]
    WDP = Ad.astype(np.float16)                                           # [128, 32]
    W2cm = W2.reshape(H1, C1, OUT_EMB).transpose(1, 0, 2).reshape(HID, OUT_EMB)
    W2e = np.concatenate([W2cm, W2cm @ att_src2.T, W2cm @ att_dst2.T], axis=1)  # [1024, 66]
    W2P = np.ascontiguousarray(
        W2e.reshape(8, 128, 66).transpose(1, 0, 2).reshape(128, 8 * 66)
    ).astype(np.float16)
    C2T = np.tile(W2e.sum(axis=0)[None, :].astype(np.float32), (128, 1))  # [128, 66]

    xt_g = np.ascontiguousarray(x.T).astype(np.float16)                   # [128, 20000]
    d_ar = np.arange(128, dtype=np.float32)

    in_maps = []
    for c in range(NC):
        no = plan['node_order'][c]
        safe = np.where(no >= 0, no, 0)
        xtp = np.ascontiguousarray(x[safe].T).astype(np.float16)          # [128, NPAD]
        idx1 = wrap_idx(plan['src_tiles'][c], NB)
        idx2 = wrap_idx(plan['node_to_row'][plan['src_tiles'][c]], NB)
        seg = plan['seg_tiles'][c]                                        # [T, 128]
        onehot = (seg[:, :, None] == d_ar[None, None, :])                 # [T, e, d]
        s01t = np.ascontiguousarray(
            onehot.transpose(1, 0, 2).reshape(128, T * 128)).astype(np.float16)
        s10t = np.ascontiguousarray(
            onehot.transpose(2, 0, 1).reshape(128, T * 128)).astype(np.float16)
        im = {
            "XT_G": xt_g, "XTP": xtp,
            "W1P": W1P, "WDP": WDP, "W2P": W2P, "C2T": C2T,
            "IDX1": idx1, "IDX2": idx2,
            "S01T": s01t, "S10T": s10t,
        }
        if np.any(b1):
            b1cm = b1.reshape(H1, C1).T.reshape(HID)
            im["B1"] = np.tile(b1cm.reshape(1, HID).astype(np.float32), (128, 1))
        if np.any(b2):
            im["B2"] = np.tile(b2.reshape(1, OUT_EMB).astype(np.float32), (128, 1))
        in_maps.append(im)

    ncb = _get_program(NB, bool(np.any(b1)), bool(np.any(b2)))

    from concourse.bass_utils import run_bass_kernel_spmd
    res = run_bass_kernel_spmd(
        ncb, in_maps, core_ids=list(range(NC)),
        trace=bool(int(os.environ.get("GAT_TRACE", "0"))),
    )

    out_full = np.zeros((N_NODES, OUT_EMB), np.float32)
    for c in range(NC):
        no = plan['node_order'][c]
        valid = no >= 0
        out_full[no[valid]] = res.results[c]["OUT"][valid]
    if _return_results:
        return out_full, res
    return out_full


# revision 36
# speedup vs baseline: 1.0753x; 1.0753x over previous
"""Two-layer GAT on 8 Trainium2 NeuronCores (Bass/Tile).

Strategy (graph/data parallel, dst-sharded):
- Host: add self-loops, sort edges by dst, shard dst-node ranges across 8
  cores, greedily pack each core's edges into 128-edge tiles grouped into
  128-node blocks (8 tiles/block); structure tables (wrapped gather idx,
  one-hot S01 [e,d] / S10 [d,e] per tile) precomputed on host.
- Device phase A (redundant on every core): HT[n] = [h1(n) | e_src1(n)]
  (fp16) for all 20000 nodes via x @ [W1|As] (TensorE), written to HBM.
  e_dst1 per packed node kept in SBUF (A2, interleaved). Layer-2 gather
  descriptors for the first blocks are pre-generated on idle GpSimd
  (SWDGE prepare_only on queues 1-3), triggered after the AllGather.
- Layer-1 edge phase, per 128-node block: one 1024-row SWDGE gather of
  HT[src]; e_dst broadcast dst->edges via S10 matmuls into PSUM;
  e = lrelu(es+ed); ex = exp(e - ln256) (shift cancels in softmax);
  messages formed in place (g1 *= ex per head); scatter-sum + ex-sum via
  S01^T matmuls into block PSUM; rinv = 1/sum ex; y = relu(o1)+exp(min(o1,0))
  (elu + 1, fp16); y^T via DMA-xbar transposes; layer-2 linear
  h2e = y^T @ W2e - colsum(W2e) (folds the elu "-1") inline; h2/es2 to
  H2TL, ed2 kept in SBUF.
- Halo exchange: AllGather of H2TL (h2 | es2 table) across 8 cores.
- Layer-2 edge phase: same machinery with H=1, C=64; gathers for blocks
  0..8 fire from pre-generated descriptors, the rest generate inline.
- Host: inverse-permute the 8 output shards into the full [20000, 64].
"""
import os
import sys
import numpy as np

sys.path.insert(0, '/opt/trn_rl_repo')

import concourse.bacc as bacc
import concourse.bass as bass
import concourse.mybir as mybir
import concourse.tile as tile
from concourse.masks import make_identity

F16 = mybir.dt.float16
F32 = mybir.dt.float32
I16 = mybir.dt.int16

N_NODES = 20000
IN_F = 128
HID = 1024          # 32 heads x 32 ch
H1, C1 = 32, 32
OUT_EMB = 64
NC = 8
SHARD = N_NODES // NC
K_TILES = 8
TILE_E = 128
NEG_SLOPE = 0.2
LN_SHIFT = float(np.log(256.0))
HT_ROW = 1152       # 1024 h + 32 es + 96 pad (2304B = 9*256)
EPS1 = float(1e-16 / 256.0)
PREP_DEPTH = 9      # layer-2 gathers pre-generated (3 SWDGE queues x 3)

# ---------------------------------------------------------------- host planning


def build_plan(edge_index: np.ndarray):
    ei = np.asarray(edge_index)
    loops = np.arange(N_NODES, dtype=ei.dtype)
    src = np.concatenate([ei[0], loops])
    dst = np.concatenate([ei[1], loops])
    order = np.argsort(dst, kind='stable')
    src_s = src[order].astype(np.int64)
    dst_s = dst[order].astype(np.int64)

    per_core = []
    max_nb = 0
    for c in range(NC):
        lo, hi = c * SHARD, (c + 1) * SHARD
        m = (dst_s >= lo) & (dst_s < hi)
        csrc, cdst = src_s[m], dst_s[m]
        nodes, starts, counts = np.unique(cdst, return_index=True, return_counts=True)
        blocks = []
        bi_nodes, bi_tiles = [], []
        t_src, t_seg = [], []

        def close_tile():
            nonlocal t_src, t_seg
            if t_src:
                bi_tiles.append((t_src, t_seg))
                t_src, t_seg = [], []

        def close_block():
            nonlocal bi_nodes, bi_tiles
            close_tile()
            if bi_nodes:
                blocks.append((bi_nodes, bi_tiles))
                bi_nodes, bi_tiles = [], []

        for n, st, cnt in zip(nodes, starts, counts):
            if len(t_src) + cnt > TILE_E:
                close_tile()
            need_new_tile = not t_src
            if len(bi_nodes) >= 128 or (need_new_tile and len(bi_tiles) >= K_TILES):
                close_block()
            local = len(bi_nodes)
            bi_nodes.append(int(n))
            t_src.extend(csrc[st:st + cnt].tolist())
            t_seg.extend([local] * int(cnt))
        close_block()
        per_core.append(blocks)
        max_nb = max(max_nb, len(blocks))

    NB = max_nb
    T = NB * K_TILES
    NPAD = NB * 128
    plan = {
        'NB': NB, 'T': T, 'NPAD': NPAD,
        'src_tiles': np.zeros((NC, T, TILE_E), np.int64),
        'seg_tiles': np.full((NC, T, TILE_E), -1.0, np.float32),
        'node_order': np.full((NC, NPAD), -1, np.int64),
    }
    for c, blocks in enumerate(per_core):
        for b, (bnodes, btiles) in enumerate(blocks):
            for r, n in enumerate(bnodes):
                plan['node_order'][c, b * 128 + r] = n
            for k, (tsrc, tseg) in enumerate(btiles):
                t = b * K_TILES + k
                plan['src_tiles'][c, t, :len(tsrc)] = tsrc
                plan['seg_tiles'][c, t, :len(tseg)] = tseg
    node_to_row = np.zeros(N_NODES, np.int64)
    for c in range(NC):
        valid = plan['node_order'][c] >= 0
        node_to_row[plan['node_order'][c][valid]] = c * NPAD + np.nonzero(valid)[0]
    plan['node_to_row'] = node_to_row
    return plan


def wrap_idx(idx_tiles: np.ndarray, NB: int) -> np.ndarray:
    """[T,128] -> dma_gather wrapped layout [128, NB*64] int16 (batch = 8 tiles)."""
    out = np.zeros((128, NB * 64), np.int16)
    flat = idx_tiles.reshape(NB, K_TILES * TILE_E)
    for b in range(NB):
        w = np.zeros((16, 64), np.int16)
        v = flat[b]
        idx = np.arange(1024)
        w[idx % 16, idx // 16] = v.astype(np.int16)
        out[:, b * 64:(b + 1) * 64] = np.tile(w, (8, 1))
    return out


# ---------------------------------------------------------------- device program

def build_program(NB, b1_nonzero, b2_nonzero, use_collective=True):
    T = NB * K_TILES
    NPAD = NB * 128
    use_mq = bool(int(os.environ.get("GAT_MQ", "1")))
    # prepare_only descriptor pre-generation deadlocks on hardware (works in
    # the scheduler sim); keep the path behind a flag, default off
    use_prep = use_mq and bool(int(os.environ.get("GAT_PREP", "0")))
    use_dmatp = bool(int(os.environ.get("GAT_DMATP", "1")))
    n_prep = min(PREP_DEPTH, NB) if use_prep else 0

    nc = bacc.Bacc("TRN2", target_bir_lowering=False, debug=False, num_devices=NC,
                   num_swdge_queues=4 if use_mq else 1,
                   dynamic_dma_scratch_size=49152 if use_prep else 32768)

    def din(name, shape, dt):
        return nc.dram_tensor(name, shape, dt, kind="ExternalInput")

    XT_G = din("XT_G", [128, N_NODES], F16)
    XTP = din("XTP", [128, NPAD], F16)
    W1P = din("W1P", [128, 1056], F16)
    WDP = din("WDP", [128, 32], F16)
    W2P = din("W2P", [128, 8 * 66], F16)
    C2T = din("C2T", [128, 66], F32)
    IDX1 = din("IDX1", [128, NB * 64], I16)
    IDX2 = din("IDX2", [128, NB * 64], I16)
    S01T = din("S01T", [128, T * 128], F16)
    S10T = din("S10T", [128, T * 128], F16)
    if b1_nonzero:
        B1 = din("B1", [128, HID], F32)
    if b2_nonzero:
        B2 = din("B2", [128, OUT_EMB], F32)

    OUT = nc.dram_tensor("OUT", [NPAD, OUT_EMB], F32, kind="ExternalOutput")

    HT = nc.dram_tensor("HT", [N_NODES, HT_ROW], F16)
    H2TL = nc.dram_tensor("H2TL", [NPAD, 128], F16)
    H2TF = nc.dram_tensor("H2TF", [NC * NPAD, 128], F16, addr_space="Shared")

    with tile.TileContext(nc) as tc:
        with (
            tc.tile_pool(name="const", bufs=1) as cpool,
            tc.tile_pool(name="sb", bufs=2) as sb,
            tc.tile_pool(name="psum", bufs=2, space="PSUM") as pp,
        ):
            # ---- resident tables
            xtp = cpool.tile([128, NPAD], F16, tag="xtp")
            nc.sync.dma_start(xtp[:], XTP[:, :])
            w1p = cpool.tile([128, 1056], F16, tag="w1p")
            nc.sync.dma_start(w1p[:], W1P[:, :])
            wdp = cpool.tile([128, 32], F16, tag="wdp")
            nc.sync.dma_start(wdp[:], WDP[:, :])
            w2p = cpool.tile([128, 8 * 66], F16, tag="w2p")
            nc.sync.dma_start(w2p[:], W2P[:, :])
            c2t = cpool.tile([128, 66], F32, tag="c2t")
            nc.sync.dma_start(c2t[:], C2T[:, :])
            idx1 = cpool.tile([128, NB * 64], I16, tag="idx1")
            nc.sync.dma_start(idx1[:], IDX1[:, :])
            idx2 = cpool.tile([128, NB * 64], I16, tag="idx2")
            nc.sync.dma_start(idx2[:], IDX2[:, :])
            if b1_nonzero:
                b1t = cpool.tile([128, HID], F32, tag="b1t")
                nc.sync.dma_start(b1t[:], B1[:, :])
            if b2_nonzero:
                b2t = cpool.tile([128, OUT_EMB], F32, tag="b2t")
                nc.sync.dma_start(b2t[:], B2[:, :])
            ed1 = cpool.tile([128, NB * 32], F16, tag="ed1")
            ed2sb = cpool.tile([128, NB], F16, tag="ed2sb")
            negshift = cpool.tile([128, 1], F32, tag="negshift")
            nc.gpsimd.memset(negshift[:], -LN_SHIFT)
            ident = cpool.tile([128, 128], F16, tag="ident")
            make_identity(nc, ident[:])

            g2sems = [nc.alloc_semaphore(f"g2dma{q}") for q in range(3)]
            g2_tiles = {}

            g2_bufs = PREP_DEPTH + 3 if use_prep else 6

            def emit_g2_gather(j, prepare):
                g2t = sb.tile([128, K_TILES, 128], F16, tag="g2",
                              bufs=g2_bufs, name=f"g2_{j}")
                g2_tiles[j] = g2t
                if prepare:
                    nc.gpsimd.dma_gather(
                        out_ap=g2t[:], in_ap=H2TF[:, :],
                        idxs_ap=idx2[:, j * 64:(j + 1) * 64],
                        num_idxs=1024, num_idxs_reg=1024, elem_size=128,
                        queue_num=1 + j % 3, prepare_only=True, sem=g2sems[j % 3])
                    return
                for hf in range(2):
                    nc.gpsimd.dma_gather(
                        out_ap=g2t[:, hf * 4:(hf + 1) * 4, :], in_ap=H2TF[:, :],
                        idxs_ap=idx2[:, j * 64 + hf * 32:j * 64 + (hf + 1) * 32],
                        num_idxs=512, num_idxs_reg=512, elem_size=128,
                        queue_num=(2 * j + hf) % 4 if use_mq else 0)

            # ================= phase A: HT = [h | es] for all nodes =========
            # (layer-2 gather preps + per-block e_dst1 (A2) interleaved)
            n_a_tiles = (N_NODES + 127) // 128
            CHUNK = 20 * 128
            prep_at = {40 + 13 * j: j for j in range(n_prep)}
            xa = None
            for m in range(n_a_tiles):
                n0 = m * 128
                nn = min(128, N_NODES - n0)
                if m % 20 == 0:
                    c0 = m * 128
                    cw = min(CHUNK, N_NODES - c0)
                    xa = sb.tile([128, CHUNK], F16, tag="xa")
                    nc.sync.dma_start(xa[:, 0:cw], XT_G[:, c0:c0 + cw])
                pa_h = pp.tile([128, 1024], F32, tag="ob")
                pa_es = pp.tile([128, 512], F32, tag="blk")
                lhsT = xa[:, n0 - (m // 20) * CHUNK:n0 - (m // 20) * CHUNK + nn]
                stage = sb.tile([128, HT_ROW], F16, tag="aStage", bufs=3)
                nc.tensor.matmul(pa_h[0:nn, 0:512], lhsT, w1p[:, 0:512], start=True, stop=True)
                nc.scalar.copy(stage[0:nn, 0:512], pa_h[0:nn, 0:512])
                nc.tensor.matmul(pa_h[0:nn, 512:1024], lhsT, w1p[:, 512:1024], start=True, stop=True)
                nc.tensor.matmul(pa_es[0:nn, 0:32], lhsT, w1p[:, 1024:1056], start=True, stop=True)
                nc.vector.tensor_copy(stage[0:nn, 512:1024], pa_h[0:nn, 512:1024])
                nc.vector.tensor_copy(stage[0:nn, 1024:1056], pa_es[0:nn, 0:32])
                nc.sync.dma_start(HT[n0:n0 + nn, :], stage[0:nn, :])
                # A2: e_dst1 per packed block, kept in SBUF
                if m % 7 == 0 and m // 7 < NB:
                    b = m // 7
                    ped = pp.tile([128, 512], F32, tag="blk")
                    nc.tensor.matmul(ped[:, 0:32], xtp[:, b * 128:(b + 1) * 128], wdp[:],
                                     start=True, stop=True)
                    nc.vector.tensor_copy(ed1[:, b * 32:(b + 1) * 32], ped[:, 0:32])
                if m in prep_at:
                    emit_g2_gather(prep_at[m], prepare=True)

            # ================= layer-1 edge phase (+ inline layer-2 linear) ==
            for b in range(NB):
                # two 512-row gathers: a full-window 1024-row gather serializes
                # desc-gen behind its own drain on the SWDGE ring
                g1a = sb.tile([128, 4, HT_ROW], F16, tag="g1", name="g1a", bufs=6)
                nc.gpsimd.dma_gather(
                    out_ap=g1a[:], in_ap=HT[:, :],
                    idxs_ap=idx1[:, b * 64:b * 64 + 32],
                    num_idxs=512, num_idxs_reg=512, elem_size=HT_ROW,
                    queue_num=(2 * b) % 4 if (use_mq and not use_prep) else 0)
                g1b = sb.tile([128, 4, HT_ROW], F16, tag="g1", name="g1b", bufs=6)
                nc.gpsimd.dma_gather(
                    out_ap=g1b[:], in_ap=HT[:, :],
                    idxs_ap=idx1[:, b * 64 + 32:(b + 1) * 64],
                    num_idxs=512, num_idxs_reg=512, elem_size=HT_ROW,
                    queue_num=(2 * b + 1) % 4 if (use_mq and not use_prep) else 0)
                g1h = [g1a, g1b]
                s01 = sb.tile([128, K_TILES, 128], F16, tag="s01")
                nc.sync.dma_start(s01[:], S01T[:, b * 1024:(b + 1) * 1024])
                s10 = sb.tile([128, K_TILES, 128], F16, tag="s10")
                nc.sync.dma_start(s10[:], S10T[:, b * 1024:(b + 1) * 1024])

                # block psum: [0:256] ed_e | [256:288] sbp | [288:354] ph
                blk = pp.tile([128, 512], F32, tag="blk")
                ed_e = blk[:, 0:256]
                for k in range(K_TILES):
                    nc.tensor.matmul(ed_e[:, k * 32:(k + 1) * 32], s10[:, k, :],
                                     ed1[:, b * 32:(b + 1) * 32],
                                     start=True, stop=True)
                # e chain (fp16)
                e_f = sb.tile([128, K_TILES * 32], F16, tag="e_f")
                for hf in range(2):
                    nc.vector.tensor_tensor(
                        out=e_f[:, hf * 128:(hf + 1) * 128]
                             .rearrange("p (k h) -> p k h", k=4),
                        in0=g1h[hf][:, :, 1024:1056],
                        in1=ed_e[:, hf * 128:(hf + 1) * 128]
                             .rearrange("p (k h) -> p k h", k=4),
                        op=mybir.AluOpType.add)
                lr = sb.tile([128, K_TILES * 32], F16, tag="lr")
                nc.vector.scalar_tensor_tensor(
                    out=lr[:], in0=e_f[:], scalar=NEG_SLOPE, in1=e_f[:],
                    op0=mybir.AluOpType.mult, op1=mybir.AluOpType.max)
                ex = sb.tile([128, K_TILES * 32], F16, tag="ex")
                nc.scalar.activation(ex[:], lr[:], mybir.ActivationFunctionType.Exp,
                                     bias=negshift[:, 0:1])
                # messages (channel-major h: inner dim head, stride-1 on both
                # DVE operands); two halves so scatter matmuls overlap
                ob = pp.tile([128, HID], F32, tag="ob")
                sbp = blk[:, 256:288]
                for hf in range(2):
                    mh = sb.tile([128, 4, HID], F16, tag="msg", name=f"msg{hf}", bufs=4)
                    nc.vector.tensor_tensor(
                        out=mh[:].rearrange("p k (c h) -> p k c h", c=C1),
                        in0=g1h[hf][:, :, 0:1024].rearrange("p k (c h) -> p k c h", c=C1),
                        in1=ex[:, hf * 128:(hf + 1) * 128]
                             .rearrange("p (k h) -> p k h", k=4)
                             .unsqueeze(2).to_broadcast([128, 4, C1, H1]),
                        op=mybir.AluOpType.mult)
                    for k in range(hf * 4, hf * 4 + 4):
                        lhsT = s01[:, k, :]
                        nc.tensor.matmul(sbp[:, 0:32], lhsT, ex[:, k * 32:(k + 1) * 32],
                                         start=(k == 0), stop=(k == K_TILES - 1))
                        nc.tensor.matmul(ob[:, 0:512], lhsT, mh[:, k % 4, 0:512],
                                         start=(k == 0), stop=(k == K_TILES - 1))
                        nc.tensor.matmul(ob[:, 512:1024], lhsT, mh[:, k % 4, 512:1024],
                                         start=(k == 0), stop=(k == K_TILES - 1))

                # ---- block finishing (fp16 elu path; y = elu + 1)
                sp = sb.tile([128, 32], F32, tag="sp")
                nc.vector.tensor_scalar_add(sp[:], sbp[:, 0:32], EPS1)
                rinv = sb.tile([128, 32], F32, tag="rinv")
                nc.vector.reciprocal(rinv[:], sp[:])
                o1h = sb.tile([128, HID], F16, tag="o1h")
                nc.vector.tensor_tensor(
                    out=o1h[:].rearrange("p (c h) -> p c h", c=C1),
                    in0=ob[:].rearrange("p (c h) -> p c h", c=C1),
                    in1=rinv[:].unsqueeze(1).to_broadcast([128, C1, H1]),
                    op=mybir.AluOpType.mult)
                if b1_nonzero:
                    nc.vector.tensor_tensor(out=o1h[:], in0=o1h[:], in1=b1t[:, :],
                                            op=mybir.AluOpType.add)
                tmin = sb.tile([128, HID], F16, tag="tmin")
                nc.vector.tensor_scalar_min(tmin[:], o1h[:], 0.0)
                nc.scalar.activation(tmin[:], tmin[:], mybir.ActivationFunctionType.Exp)
                yb = sb.tile([128, HID], F16, tag="yb")
                nc.vector.scalar_tensor_tensor(
                    out=yb[:], in0=o1h[:], scalar=0.0, in1=tmin[:],
                    op0=mybir.AluOpType.max, op1=mybir.AluOpType.add)
                # y^T via PE transposes into one f16 psum bank
                tpb = pp.tile([128, HID], F16, tag="tp", bufs=2)
                for kk in range(8):
                    nc.tensor.transpose(tpb[:, kk * 128:(kk + 1) * 128],
                                        yb[:, kk * 128:(kk + 1) * 128], ident[:])
                tstage = sb.tile([128, HID], F16, tag="tst")
                nc.scalar.copy(tstage[:], tpb[:])
                # layer-2 linear: h2e = y^T @ W2e - colsum(W2e)  (elu -1 folded)
                ph = blk[:, 288:354]
                for k in range(8):
                    nc.tensor.matmul(ph[:, 0:66], tstage[:, k * 128:(k + 1) * 128],
                                     w2p[:, k * 66:(k + 1) * 66],
                                     start=(k == 0), stop=(k == 7))
                h2s = sb.tile([128, 128], F16, tag="h2s")
                nc.vector.tensor_tensor(out=h2s[:, 0:65], in0=ph[:, 0:65],
                                        in1=c2t[:, 0:65], op=mybir.AluOpType.subtract)
                nc.vector.tensor_tensor(out=ed2sb[:, b:b + 1], in0=ph[:, 65:66],
                                        in1=c2t[:, 65:66], op=mybir.AluOpType.subtract)
                nc.sync.dma_start(H2TL[b * 128:(b + 1) * 128, :], h2s[:])

            # ================= halo exchange =================
            if use_collective:
                nc.gpsimd.collective_compute(
                    "AllGather",
                    mybir.AluOpType.bypass,
                    ins=[H2TL.ap().opt()],
                    outs=[H2TF.ap().opt()],
                    replica_groups=[list(range(NC))],
                )
            else:
                nc.sync.dma_start(H2TF[0:NPAD, :], H2TL[:, :])

            # ================= layer-2 edge phase =================
            for j in range(n_prep):
                nc.gpsimd.trigger_dma(count=1, queue_num=1 + j % 3)
            for b in range(NB):
                if b + n_prep < NB:
                    emit_g2_gather(b + n_prep, prepare=False)
                g2 = g2_tiles[b]
                s01b = sb.tile([128, K_TILES, 128], F16, tag="s01L2", bufs=3)
                nc.sync.dma_start(s01b[:], S01T[:, b * 1024:(b + 1) * 1024])
                s10b = sb.tile([128, K_TILES, 128], F16, tag="s10L2", bufs=3)
                nc.sync.dma_start(s10b[:], S10T[:, b * 1024:(b + 1) * 1024])

                blk2 = pp.tile([128, 512], F32, tag="blk")
                ed2_e = blk2[:, 0:256]
                for k in range(K_TILES):
                    nc.tensor.matmul(ed2_e[:, k:k + 1], s10b[:, k, :],
                                     ed2sb[:, b:b + 1], start=True, stop=True)
                e2 = sb.tile([128, K_TILES], F16, tag="e2", bufs=3)
                nc.vector.tensor_tensor(
                    out=e2[:].unsqueeze(2),
                    in0=g2[:, :, 64:65],
                    in1=ed2_e[:, 0:K_TILES].unsqueeze(2),
                    op=mybir.AluOpType.add)
                nc.vector.scalar_tensor_tensor(
                    out=e2[:], in0=e2[:], scalar=NEG_SLOPE, in1=e2[:],
                    op0=mybir.AluOpType.mult, op1=mybir.AluOpType.max)
                ex2 = sb.tile([128, K_TILES], F16, tag="ex2", bufs=3)
                nc.scalar.activation(ex2[:], e2[:], mybir.ActivationFunctionType.Exp,
                                     bias=negshift[:, 0:1])
                msg2 = sb.tile([128, K_TILES, OUT_EMB], F16, tag="msg2", bufs=3)
                nc.vector.tensor_tensor(
                    out=msg2[:],
                    in0=g2[:, :, 0:64],
                    in1=ex2[:].unsqueeze(2).to_broadcast([128, K_TILES, OUT_EMB]),
                    op=mybir.AluOpType.mult)
                ob2 = pp.tile([128, HID], F32, tag="ob")
                sb2 = blk2[:, 256:288]
                for k in range(K_TILES):
                    lhsT = s01b[:, k, :]
                    nc.tensor.matmul(sb2[:, 0:1], lhsT, ex2[:, k:k + 1],
                                     start=(k == 0), stop=(k == K_TILES - 1))
                    nc.tensor.matmul(ob2[:, 0:64], lhsT, msg2[:, k, :],
                                     start=(k == 0), stop=(k == K_TILES - 1))
                sp2 = sb.tile([128, 1], F32, tag="sp2", bufs=3)
                nc.vector.tensor_scalar_add(sp2[:], sb2[:, 0:1], EPS1)
                rinv2 = sb.tile([128, 1], F32, tag="rinv2", bufs=3)
                nc.vector.reciprocal(rinv2[:], sp2[:])
                o2 = sb.tile([128, OUT_EMB], F32, tag="o2", bufs=3)
                nc.vector.tensor_scalar_mul(o2[:], ob2[:, 0:64], rinv2[:, 0:1])
                if b2_nonzero:
                    nc.vector.tensor_tensor(out=o2[:], in0=o2[:], in1=b2t[:, :],
                                            op=mybir.AluOpType.add)
                nc.sync.dma_start(OUT[b * 128:(b + 1) * 128, :], o2[:])

    nc.compile()
    return nc


# ---------------------------------------------------------------- driver

_CACHE = {}


def _get_program(NB, b1_nonzero, b2_nonzero):
    key = (NB, b1_nonzero, b2_nonzero)
    if key not in _CACHE:
        _CACHE[key] = build_program(NB, b1_nonzero, b2_nonzero)
    return _CACHE[key]


def kernel(x, edge_index, W1, att_src1, att_dst1, b1, W2, att_src2, att_dst2, b2,
           _return_results=False):
    x = np.asarray(x); edge_index = np.asarray(edge_index)
    W1 = np.asarray(W1); att_src1 = np.asarray(att_src1); att_dst1 = np.asarray(att_dst1)
    b1 = np.asarray(b1); W2 = np.asarray(W2)
    att_src2 = np.asarray(att_src2); att_dst2 = np.asarray(att_dst2); b2 = np.asarray(b2)

    plan = build_plan(edge_index)
    NB, T, NPAD = plan['NB'], plan['T'], plan['NPAD']

    # fused weights (host, fp32 math then fp16); hidden in channel-major
    # layout (col c*32+h) so the per-head DVE broadcasts keep stride-1 inner
    W1r = W1.reshape(IN_F, H1, C1)
    As = np.einsum('fhc,hc->fh', W1r, att_src1)
    Ad = np.einsum('fhc,hc->fh', W1r, att_dst1)
    W1cm = W1r.transpose(0, 2, 1).reshape(IN_F, HID)
    W1P = np.concatenate([W1cm, As], axis=1).astype(np.float16)           # [128, 1056]
    WDP = Ad.astype(np.float16)                                           # [128, 32]
    W2cm = W2.reshape(H1, C1, OUT_EMB).transpose(1, 0, 2).reshape(HID, OUT_EMB)
    W2e = np.concatenate([W2cm, W2cm @ att_src2.T, W2cm @ att_dst2.T], axis=1)  # [1024, 66]
    W2P = np.ascontiguousarray(
        W2e.reshape(8, 128, 66).transpose(1, 0, 2).reshape(128, 8 * 66)
    ).astype(np.float16)
    C2T = np.tile(W2e.sum(axis=0)[None, :].astype(np.float32), (128, 1))  # [128, 66]

    xt_g = np.ascontiguousarray(x.T).astype(np.float16)                   # [128, 20000]
    d_ar = np.arange(128, dtype=np.float32)

    in_maps = []
    for c in range(NC):
        no = plan['node_order'][c]
        safe = np.where(no >= 0, no, 0)
        xtp = np.ascontiguousarray(x[safe].T).astype(np.float16)          # [128, NPAD]
        idx1 = wrap_idx(plan['src_tiles'][c], NB)
        idx2 = wrap_idx(plan['node_to_row'][plan['src_tiles'][c]], NB)
        seg = plan['seg_tiles'][c]                                        # [T, 128]
        onehot = (seg[:, :, None] == d_ar[None, None, :])                 # [T, e, d]
        s01t = np.ascontiguousarray(
            onehot.transpose(1, 0, 2).reshape(128, T * 128)).astype(np.float16)
        s10t = np.ascontiguousarray(
            onehot.transpose(2, 0, 1).reshape(128, T * 128)).astype(np.float16)
        im = {
            "XT_G": xt_g, "XTP": xtp,
            "W1P": W1P, "WDP": WDP, "W2P": W2P, "C2T": C2T,
            "IDX1": idx1, "IDX2": idx2,
            "S01T": s01t, "S10T": s10t,
        }
        if np.any(b1):
            b1cm = b1.reshape(H1, C1).T.reshape(HID)
            im["B1"] = np.tile(b1cm.reshape(1, HID).astype(np.float32), (128, 1))
        if np.any(b2):
            im["B2"] = np.tile(b2.reshape(1, OUT_EMB).astype(np.float32), (128, 1))
        in_maps.append(im)

    ncb = _get_program(NB, bool(np.any(b1)), bool(np.any(b2)))

    from concourse.bass_utils import run_bass_kernel_spmd
    res = run_bass_kernel_spmd(
        ncb, in_maps, core_ids=list(range(NC)),
        trace=bool(int(os.environ.get("GAT_TRACE", "0"))),
    )

    out_full = np.zeros((N_NODES, OUT_EMB), np.float32)
    for c in range(NC):
        no = plan['node_order'][c]
        valid = no >= 0
        out_full[no[valid]] = res.results[c]["OUT"][valid]
    if _return_results:
        return out_full, res
    return out_full


# revision 47
# speedup vs baseline: 1.3074x; 1.2159x over previous
"""Two-layer GAT on 8 Trainium2 NeuronCores (Bass/Tile).

Strategy (graph/data parallel, dst-sharded):
- Host: add self-loops, sort edges by dst, shard dst-node ranges across 8
  cores, greedily pack each core's edges into 128-edge tiles grouped into
  128-node blocks (8 tiles/block); structure tables (wrapped gather idx,
  one-hot S01 [e,d] / S10 [d,e] per tile) precomputed on host.
- Device phase A (redundant on every core): HT[n] = [h1(n) | e_src1(n)]
  (fp16) for all 20000 nodes via x @ [W1|As] (TensorE), written to HBM.
  e_dst1 per packed node kept in SBUF (A2, interleaved). Layer-2 gather
  descriptors for the first blocks are pre-generated on idle GpSimd
  (SWDGE prepare_only on queues 1-3), triggered after the AllGather.
- Layer-1 edge phase, per 128-node block: one 1024-row SWDGE gather of
  HT[src]; e_dst broadcast dst->edges via S10 matmuls into PSUM;
  e = lrelu(es+ed); ex = exp(e - ln256) (shift cancels in softmax);
  messages formed in place (g1 *= ex per head); scatter-sum + ex-sum via
  S01^T matmuls into block PSUM; rinv = 1/sum ex; y = relu(o1)+exp(min(o1,0))
  (elu + 1, fp16); y^T via DMA-xbar transposes; layer-2 linear
  h2e = y^T @ W2e - colsum(W2e) (folds the elu "-1") inline; h2/es2 to
  H2TL, ed2 kept in SBUF.
- Halo exchange: AllGather of H2TL (h2 | es2 table) across 8 cores.
- Layer-2 edge phase: same machinery with H=1, C=64; gathers for blocks
  0..8 fire from pre-generated descriptors, the rest generate inline.
- Host: inverse-permute the 8 output shards into the full [20000, 64].
"""
import os
import sys
import numpy as np

sys.path.insert(0, '/opt/trn_rl_repo')

import concourse.bacc as bacc
import concourse.bass as bass
import concourse.mybir as mybir
import concourse.tile as tile
from concourse.masks import make_identity

F16 = mybir.dt.float16
F32 = mybir.dt.float32
I16 = mybir.dt.int16

N_NODES = 20000
IN_F = 128
HID = 1024          # 32 heads x 32 ch
H1, C1 = 32, 32
OUT_EMB = 64
NC = 8
SHARD = N_NODES // NC
K_TILES = 8
TILE_E = 128
NEG_SLOPE = 0.2
LN_SHIFT = float(np.log(256.0))
HT_ROW = 1152       # 1024 h + 32 es + 96 pad (2304B = 9*256)
EPS1 = float(1e-16 / 256.0)
PREP_DEPTH = 9      # layer-2 gathers pre-generated (3 SWDGE queues x 3)

# ---------------------------------------------------------------- host planning


def build_plan(edge_index: np.ndarray):
    ei = np.asarray(edge_index)
    loops = np.arange(N_NODES, dtype=ei.dtype)
    src = np.concatenate([ei[0], loops])
    dst = np.concatenate([ei[1], loops])
    order = np.argsort(dst, kind='stable')
    src_s = src[order].astype(np.int64)
    dst_s = dst[order].astype(np.int64)

    per_core = []
    max_nb = 0
    for c in range(NC):
        lo, hi = c * SHARD, (c + 1) * SHARD
        m = (dst_s >= lo) & (dst_s < hi)
        csrc, cdst = src_s[m], dst_s[m]
        nodes, starts, counts = np.unique(cdst, return_index=True, return_counts=True)
        blocks = []
        bi_nodes, bi_tiles = [], []
        t_src, t_seg = [], []

        def close_tile():
            nonlocal t_src, t_seg
            if t_src:
                bi_tiles.append((t_src, t_seg))
                t_src, t_seg = [], []

        def close_block():
            nonlocal bi_nodes, bi_tiles
            close_tile()
            if bi_nodes:
                blocks.append((bi_nodes, bi_tiles))
                bi_nodes, bi_tiles = [], []

        for n, st, cnt in zip(nodes, starts, counts):
            if len(t_src) + cnt > TILE_E:
                close_tile()
            need_new_tile = not t_src
            if len(bi_nodes) >= 128 or (need_new_tile and len(bi_tiles) >= K_TILES):
                close_block()
            local = len(bi_nodes)
            bi_nodes.append(int(n))
            t_src.extend(csrc[st:st + cnt].tolist())
            t_seg.extend([local] * int(cnt))
        close_block()
        per_core.append(blocks)
        max_nb = max(max_nb, len(blocks))

    NB = max_nb
    T = NB * K_TILES
    NPAD = NB * 128
    plan = {
        'NB': NB, 'T': T, 'NPAD': NPAD,
        'src_tiles': np.zeros((NC, T, TILE_E), np.int64),
        'seg_tiles': np.full((NC, T, TILE_E), -1.0, np.float32),
        'node_order': np.full((NC, NPAD), -1, np.int64),
    }
    for c, blocks in enumerate(per_core):
        for b, (bnodes, btiles) in enumerate(blocks):
            for r, n in enumerate(bnodes):
                plan['node_order'][c, b * 128 + r] = n
            for k, (tsrc, tseg) in enumerate(btiles):
                t = b * K_TILES + k
                plan['src_tiles'][c, t, :len(tsrc)] = tsrc
                plan['seg_tiles'][c, t, :len(tseg)] = tseg
    node_to_row = np.zeros(N_NODES, np.int64)
    for c in range(NC):
        valid = plan['node_order'][c] >= 0
        node_to_row[plan['node_order'][c][valid]] = c * NPAD + np.nonzero(valid)[0]
    plan['node_to_row'] = node_to_row
    return plan


def wrap_idx(idx_tiles: np.ndarray, NB: int) -> np.ndarray:
    """[T,128] -> dma_gather wrapped layout [128, NB*64] int16 (batch = 8 tiles)."""
    out = np.zeros((128, NB * 64), np.int16)
    flat = idx_tiles.reshape(NB, K_TILES * TILE_E)
    for b in range(NB):
        w = np.zeros((16, 64), np.int16)
        v = flat[b]
        idx = np.arange(1024)
        w[idx % 16, idx // 16] = v.astype(np.int16)
        out[:, b * 64:(b + 1) * 64] = np.tile(w, (8, 1))
    return out


# ---------------------------------------------------------------- device program

def build_program(NB, MT, b1_nonzero, b2_nonzero, use_collective=True):
    T = NB * K_TILES
    NPAD = NB * 128
    NLOC = MT * 128     # per-core distinct-source table rows (padded)
    use_mq = bool(int(os.environ.get("GAT_MQ", "1")))
    # prepare_only descriptor pre-generation deadlocks on hardware (works in
    # the scheduler sim); keep the path behind a flag, default off
    use_prep = use_mq and bool(int(os.environ.get("GAT_PREP", "0")))
    use_dmatp = bool(int(os.environ.get("GAT_DMATP", "1")))
    n_prep = min(PREP_DEPTH, NB) if use_prep else 0

    nc = bacc.Bacc("TRN2", target_bir_lowering=False, debug=False, num_devices=NC,
                   num_swdge_queues=4 if use_mq else 1,
                   dynamic_dma_scratch_size=49152 if use_prep else 32768)

    def din(name, shape, dt):
        return nc.dram_tensor(name, shape, dt, kind="ExternalInput")

    XT_G = din("XT_G", [128, NLOC], F16)
    XTP = din("XTP", [128, NPAD], F16)
    W1P = din("W1P", [128, 1056], F16)
    WDP = din("WDP", [128, 32], F16)
    W2P = din("W2P", [128, 8 * 66], F16)
    C2T = din("C2T", [128, 66], F32)
    IDX1 = din("IDX1", [128, NB * 64], I16)
    IDX2 = din("IDX2", [128, NB * 64], I16)
    S01T = din("S01T", [128, T * 128], F16)
    S10T = din("S10T", [128, T * 128], F16)
    if b1_nonzero:
        B1 = din("B1", [128, HID], F32)
    if b2_nonzero:
        B2 = din("B2", [128, OUT_EMB], F32)

    OUT = nc.dram_tensor("OUT", [NPAD, OUT_EMB], F32, kind="ExternalOutput")

    HT = nc.dram_tensor("HT", [NLOC, HT_ROW], F16)
    H2TL = nc.dram_tensor("H2TL", [NPAD, 128], F16)
    H2TF = nc.dram_tensor("H2TF", [NC * NPAD, 128], F16, addr_space="Shared")

    with tile.TileContext(nc) as tc:
        with (
            tc.tile_pool(name="const", bufs=1) as cpool,
            tc.tile_pool(name="sb", bufs=2) as sb,
            tc.tile_pool(name="psum", bufs=2, space="PSUM") as pp,
        ):
            # ---- resident tables
            xtp = cpool.tile([128, NPAD], F16, tag="xtp")
            nc.sync.dma_start(xtp[:], XTP[:, :])
            w1p = cpool.tile([128, 1056], F16, tag="w1p")
            nc.sync.dma_start(w1p[:], W1P[:, :])
            wdp = cpool.tile([128, 32], F16, tag="wdp")
            nc.sync.dma_start(wdp[:], WDP[:, :])
            w2p = cpool.tile([128, 8 * 66], F16, tag="w2p")
            nc.sync.dma_start(w2p[:], W2P[:, :])
            c2t = cpool.tile([128, 66], F32, tag="c2t")
            nc.sync.dma_start(c2t[:], C2T[:, :])
            idx1 = cpool.tile([128, NB * 64], I16, tag="idx1")
            nc.sync.dma_start(idx1[:], IDX1[:, :])
            idx2 = cpool.tile([128, NB * 64], I16, tag="idx2")
            nc.sync.dma_start(idx2[:], IDX2[:, :])
            if b1_nonzero:
                b1t = cpool.tile([128, HID], F32, tag="b1t")
                nc.sync.dma_start(b1t[:], B1[:, :])
            if b2_nonzero:
                b2t = cpool.tile([128, OUT_EMB], F32, tag="b2t")
                nc.sync.dma_start(b2t[:], B2[:, :])
            ed1 = cpool.tile([128, NB * 32], F16, tag="ed1")
            ed2sb = cpool.tile([128, NB], F16, tag="ed2sb")
            negshift = cpool.tile([128, 1], F32, tag="negshift")
            nc.gpsimd.memset(negshift[:], -LN_SHIFT)
            ident = cpool.tile([128, 128], F16, tag="ident")
            make_identity(nc, ident[:])

            g2sems = [nc.alloc_semaphore(f"g2dma{q}") for q in range(3)]
            g2_tiles = {}

            g2_bufs = PREP_DEPTH + 3 if use_prep else 6

            def emit_g2_gather(j, prepare):
                g2t = sb.tile([128, K_TILES, 128], F16, tag="g2",
                              bufs=g2_bufs, name=f"g2_{j}")
                g2_tiles[j] = g2t
                if prepare:
                    nc.gpsimd.dma_gather(
                        out_ap=g2t[:], in_ap=H2TF[:, :],
                        idxs_ap=idx2[:, j * 64:(j + 1) * 64],
                        num_idxs=1024, num_idxs_reg=1024, elem_size=128,
                        queue_num=1 + j % 3, prepare_only=True, sem=g2sems[j % 3])
                    return
                for hf in range(2):
                    nc.gpsimd.dma_gather(
                        out_ap=g2t[:, hf * 4:(hf + 1) * 4, :], in_ap=H2TF[:, :],
                        idxs_ap=idx2[:, j * 64 + hf * 32:j * 64 + (hf + 1) * 32],
                        num_idxs=512, num_idxs_reg=512, elem_size=128,
                        queue_num=(2 * j + hf) % 4 if use_mq else 0)

            # ================= phase A: HT = [h | es] for this core's
            # distinct sources only (host-renumbered); layer-2 gather preps
            # + per-block e_dst1 (A2) interleaved
            n_a_tiles = MT
            CHUNK = 20 * 128
            prep_spacing = max(1, (n_a_tiles - 20) // max(n_prep, 1))
            prep_at = {20 + prep_spacing * j: j for j in range(n_prep)}
            a2_every = max(1, n_a_tiles // NB)
            xa = None
            for m in range(n_a_tiles):
                n0 = m * 128
                nn = 128
                if m % 20 == 0:
                    c0 = m * 128
                    cw = min(CHUNK, NLOC - c0)
                    xa = sb.tile([128, CHUNK], F16, tag="xa")
                    nc.sync.dma_start(xa[:, 0:cw], XT_G[:, c0:c0 + cw])
                pa_h = pp.tile([128, 1024], F32, tag="ob")
                pa_es = pp.tile([128, 512], F32, tag="blk")
                lhsT = xa[:, n0 - (m // 20) * CHUNK:n0 - (m // 20) * CHUNK + nn]
                stage = sb.tile([128, HT_ROW], F16, tag="aStage", bufs=3)
                nc.tensor.matmul(pa_h[0:nn, 0:512], lhsT, w1p[:, 0:512], start=True, stop=True)
                nc.scalar.copy(stage[0:nn, 0:512], pa_h[0:nn, 0:512])
                nc.tensor.matmul(pa_h[0:nn, 512:1024], lhsT, w1p[:, 512:1024], start=True, stop=True)
                nc.tensor.matmul(pa_es[0:nn, 0:32], lhsT, w1p[:, 1024:1056], start=True, stop=True)
                nc.vector.tensor_copy(stage[0:nn, 512:1024], pa_h[0:nn, 512:1024])
                nc.vector.tensor_copy(stage[0:nn, 1024:1056], pa_es[0:nn, 0:32])
                nc.sync.dma_start(HT[n0:n0 + nn, :], stage[0:nn, :])
                # A2: e_dst1 per packed block, kept in SBUF
                if m % a2_every == 0 and m // a2_every < NB:
                    b = m // a2_every
                    ped = pp.tile([128, 512], F32, tag="blk")
                    nc.tensor.matmul(ped[:, 0:32], xtp[:, b * 128:(b + 1) * 128], wdp[:],
                                     start=True, stop=True)
                    nc.vector.tensor_copy(ed1[:, b * 32:(b + 1) * 32], ped[:, 0:32])
                if m in prep_at:
                    emit_g2_gather(prep_at[m], prepare=True)

            # ================= layer-1 edge phase (+ inline layer-2 linear) ==
            for b in range(NB):
                # two 512-row gathers: a full-window 1024-row gather serializes
                # desc-gen behind its own drain on the SWDGE ring
                g1a = sb.tile([128, 4, HT_ROW], F16, tag="g1", name="g1a", bufs=6)
                nc.gpsimd.dma_gather(
                    out_ap=g1a[:], in_ap=HT[:, :],
                    idxs_ap=idx1[:, b * 64:b * 64 + 32],
                    num_idxs=512, num_idxs_reg=512, elem_size=HT_ROW,
                    queue_num=(2 * b) % 4 if (use_mq and not use_prep) else 0)
                g1b = sb.tile([128, 4, HT_ROW], F16, tag="g1", name="g1b", bufs=6)
                nc.gpsimd.dma_gather(
                    out_ap=g1b[:], in_ap=HT[:, :],
                    idxs_ap=idx1[:, b * 64 + 32:(b + 1) * 64],
                    num_idxs=512, num_idxs_reg=512, elem_size=HT_ROW,
                    queue_num=(2 * b + 1) % 4 if (use_mq and not use_prep) else 0)
                g1h = [g1a, g1b]
                s01 = sb.tile([128, K_TILES, 128], F16, tag="s01")
                nc.sync.dma_start(s01[:], S01T[:, b * 1024:(b + 1) * 1024])
                s10 = sb.tile([128, K_TILES, 128], F16, tag="s10")
                nc.sync.dma_start(s10[:], S10T[:, b * 1024:(b + 1) * 1024])

                # block psum: [0:256] ed_e | [256:288] sbp | [288:354] ph
                blk = pp.tile([128, 512], F32, tag="blk")
                ed_e = blk[:, 0:256]
                for k in range(K_TILES):
                    nc.tensor.matmul(ed_e[:, k * 32:(k + 1) * 32], s10[:, k, :],
                                     ed1[:, b * 32:(b + 1) * 32],
                                     start=True, stop=True)
                # e chain (fp16)
                e_f = sb.tile([128, K_TILES * 32], F16, tag="e_f")
                for hf in range(2):
                    nc.vector.tensor_tensor(
                        out=e_f[:, hf * 128:(hf + 1) * 128]
                             .rearrange("p (k h) -> p k h", k=4),
                        in0=g1h[hf][:, :, 1024:1056],
                        in1=ed_e[:, hf * 128:(hf + 1) * 128]
                             .rearrange("p (k h) -> p k h", k=4),
                        op=mybir.AluOpType.add)
                lr = sb.tile([128, K_TILES * 32], F16, tag="lr")
                nc.vector.scalar_tensor_tensor(
                    out=lr[:], in0=e_f[:], scalar=NEG_SLOPE, in1=e_f[:],
                    op0=mybir.AluOpType.mult, op1=mybir.AluOpType.max)
                ex = sb.tile([128, K_TILES * 32], F16, tag="ex")
                nc.scalar.activation(ex[:], lr[:], mybir.ActivationFunctionType.Exp,
                                     bias=negshift[:, 0:1])
                # messages (channel-major h: inner dim head, stride-1 on both
                # DVE operands); two halves so scatter matmuls overlap
                ob = pp.tile([128, HID], F32, tag="ob")
                sbp = blk[:, 256:288]
                for hf in range(2):
                    mh = sb.tile([128, 4, HID], F16, tag="msg", name=f"msg{hf}", bufs=4)
                    nc.vector.tensor_tensor(
                        out=mh[:].rearrange("p k (c h) -> p k c h", c=C1),
                        in0=g1h[hf][:, :, 0:1024].rearrange("p k (c h) -> p k c h", c=C1),
                        in1=ex[:, hf * 128:(hf + 1) * 128]
                             .rearrange("p (k h) -> p k h", k=4)
                             .unsqueeze(2).to_broadcast([128, 4, C1, H1]),
                        op=mybir.AluOpType.mult)
                    for k in range(hf * 4, hf * 4 + 4):
                        lhsT = s01[:, k, :]
                        nc.tensor.matmul(sbp[:, 0:32], lhsT, ex[:, k * 32:(k + 1) * 32],
                                         start=(k == 0), stop=(k == K_TILES - 1))
                        nc.tensor.matmul(ob[:, 0:512], lhsT, mh[:, k % 4, 0:512],
                                         start=(k == 0), stop=(k == K_TILES - 1))
                        nc.tensor.matmul(ob[:, 512:1024], lhsT, mh[:, k % 4, 512:1024],
                                         start=(k == 0), stop=(k == K_TILES - 1))

                # ---- block finishing (fp16 elu path; y = elu + 1)
                sp = sb.tile([128, 32], F32, tag="sp")
                nc.vector.tensor_scalar_add(sp[:], sbp[:, 0:32], EPS1)
                rinv = sb.tile([128, 32], F32, tag="rinv")
                nc.vector.reciprocal(rinv[:], sp[:])
                o1h = sb.tile([128, HID], F16, tag="o1h")
                nc.vector.tensor_tensor(
                    out=o1h[:].rearrange("p (c h) -> p c h", c=C1),
                    in0=ob[:].rearrange("p (c h) -> p c h", c=C1),
                    in1=rinv[:].unsqueeze(1).to_broadcast([128, C1, H1]),
                    op=mybir.AluOpType.mult)
                if b1_nonzero:
                    nc.vector.tensor_tensor(out=o1h[:], in0=o1h[:], in1=b1t[:, :],
                                            op=mybir.AluOpType.add)
                # y = elu(x)+1 = relu(x) + min(exp(x), 1): exact (exp overflow
                # saturates to inf, min clamps); single DVE pass after two
                # scalar-engine activations
                texp = sb.tile([128, HID], F16, tag="texp")
                nc.scalar.activation(texp[:], o1h[:], mybir.ActivationFunctionType.Exp)
                trel = sb.tile([128, HID], F16, tag="trel")
                nc.scalar.activation(trel[:], o1h[:], mybir.ActivationFunctionType.Relu)
                yb = sb.tile([128, HID], F16, tag="yb")
                nc.vector.scalar_tensor_tensor(
                    out=yb[:], in0=texp[:], scalar=1.0, in1=trel[:],
                    op0=mybir.AluOpType.min, op1=mybir.AluOpType.add)
                # y^T via PE transposes into one f16 psum bank
                tpb = pp.tile([128, HID], F16, tag="tp", bufs=2)
                for kk in range(8):
                    nc.tensor.transpose(tpb[:, kk * 128:(kk + 1) * 128],
                                        yb[:, kk * 128:(kk + 1) * 128], ident[:])
                tstage = sb.tile([128, HID], F16, tag="tst")
                nc.scalar.copy(tstage[:], tpb[:])
                # layer-2 linear: h2e = y^T @ W2e - colsum(W2e)  (elu -1 folded)
                ph = blk[:, 288:354]
                for k in range(8):
                    nc.tensor.matmul(ph[:, 0:66], tstage[:, k * 128:(k + 1) * 128],
                                     w2p[:, k * 66:(k + 1) * 66],
                                     start=(k == 0), stop=(k == 7))
                h2s = sb.tile([128, 128], F16, tag="h2s")
                nc.vector.tensor_tensor(out=h2s[:, 0:65], in0=ph[:, 0:65],
                                        in1=c2t[:, 0:65], op=mybir.AluOpType.subtract)
                nc.vector.tensor_tensor(out=ed2sb[:, b:b + 1], in0=ph[:, 65:66],
                                        in1=c2t[:, 65:66], op=mybir.AluOpType.subtract)
                nc.sync.dma_start(H2TL[b * 128:(b + 1) * 128, :], h2s[:])

            # ================= halo exchange =================
            if use_collective:
                nc.gpsimd.collective_compute(
                    "AllGather",
                    mybir.AluOpType.bypass,
                    ins=[H2TL.ap().opt()],
                    outs=[H2TF.ap().opt()],
                    replica_groups=[list(range(NC))],
                )
            else:
                nc.sync.dma_start(H2TF[0:NPAD, :], H2TL[:, :])

            # ================= layer-2 edge phase =================
            for j in range(n_prep):
                nc.gpsimd.trigger_dma(count=1, queue_num=1 + j % 3)
            for b in range(NB):
                if b + n_prep < NB:
                    emit_g2_gather(b + n_prep, prepare=False)
                g2 = g2_tiles[b]
                s01b = sb.tile([128, K_TILES, 128], F16, tag="s01L2", bufs=3)
                nc.sync.dma_start(s01b[:], S01T[:, b * 1024:(b + 1) * 1024])
                s10b = sb.tile([128, K_TILES, 128], F16, tag="s10L2", bufs=3)
                nc.sync.dma_start(s10b[:], S10T[:, b * 1024:(b + 1) * 1024])

                blk2 = pp.tile([128, 512], F32, tag="blk")
                ed2_e = blk2[:, 0:256]
                for k in range(K_TILES):
                    nc.tensor.matmul(ed2_e[:, k:k + 1], s10b[:, k, :],
                                     ed2sb[:, b:b + 1], start=True, stop=True)
                e2 = sb.tile([128, K_TILES], F16, tag="e2", bufs=3)
                nc.vector.tensor_tensor(
                    out=e2[:].unsqueeze(2),
                    in0=g2[:, :, 64:65],
                    in1=ed2_e[:, 0:K_TILES].unsqueeze(2),
                    op=mybir.AluOpType.add)
                nc.vector.scalar_tensor_tensor(
                    out=e2[:], in0=e2[:], scalar=NEG_SLOPE, in1=e2[:],
                    op0=mybir.AluOpType.mult, op1=mybir.AluOpType.max)
                ex2 = sb.tile([128, K_TILES], F16, tag="ex2", bufs=3)
                nc.scalar.activation(ex2[:], e2[:], mybir.ActivationFunctionType.Exp,
                                     bias=negshift[:, 0:1])
                msg2 = sb.tile([128, K_TILES, OUT_EMB], F16, tag="msg2", bufs=3)
                nc.vector.tensor_tensor(
                    out=msg2[:],
                    in0=g2[:, :, 0:64],
                    in1=ex2[:].unsqueeze(2).to_broadcast([128, K_TILES, OUT_EMB]),
                    op=mybir.AluOpType.mult)
                ob2 = pp.tile([128, HID], F32, tag="ob")
                sb2 = blk2[:, 256:288]
                for k in range(K_TILES):
                    lhsT = s01b[:, k, :]
                    nc.tensor.matmul(sb2[:, 0:1], lhsT, ex2[:, k:k + 1],
                                     start=(k == 0), stop=(k == K_TILES - 1))
                    nc.tensor.matmul(ob2[:, 0:64], lhsT, msg2[:, k, :],
                                     start=(k == 0), stop=(k == K_TILES - 1))
                sp2 = sb.tile([128, 1], F32, tag="sp2", bufs=3)
                nc.vector.tensor_scalar_add(sp2[:], sb2[:, 0:1], EPS1)
                rinv2 = sb.tile([128, 1], F32, tag="rinv2", bufs=3)
                nc.vector.reciprocal(rinv2[:], sp2[:])
                o2 = sb.tile([128, OUT_EMB], F32, tag="o2", bufs=3)
                nc.vector.tensor_scalar_mul(o2[:], ob2[:, 0:64], rinv2[:, 0:1])
                if b2_nonzero:
                    nc.vector.tensor_tensor(out=o2[:], in0=o2[:], in1=b2t[:, :],
                                            op=mybir.AluOpType.add)
                nc.sync.dma_start(OUT[b * 128:(b + 1) * 128, :], o2[:])

    nc.compile()
    return nc


# ---------------------------------------------------------------- driver

_CACHE = {}


def _get_program(NB, MT, b1_nonzero, b2_nonzero):
    key = (NB, MT, b1_nonzero, b2_nonzero)
    if key not in _CACHE:
        _CACHE[key] = build_program(NB, MT, b1_nonzero, b2_nonzero)
    return _CACHE[key]


def kernel(x, edge_index, W1, att_src1, att_dst1, b1, W2, att_src2, att_dst2, b2,
           _return_results=False):
    x = np.asarray(x); edge_index = np.asarray(edge_index)
    W1 = np.asarray(W1); att_src1 = np.asarray(att_src1); att_dst1 = np.asarray(att_dst1)
    b1 = np.asarray(b1); W2 = np.asarray(W2)
    att_src2 = np.asarray(att_src2); att_dst2 = np.asarray(att_dst2); b2 = np.asarray(b2)

    plan = build_plan(edge_index)
    NB, T, NPAD = plan['NB'], plan['T'], plan['NPAD']

    # fused weights (host, fp32 math then fp16); hidden in channel-major
    # layout (col c*32+h) so the per-head DVE broadcasts keep stride-1 inner
    W1r = W1.reshape(IN_F, H1, C1)
    As = np.einsum('fhc,hc->fh', W1r, att_src1)
    Ad = np.einsum('fhc,hc->fh', W1r, att_dst1)
    W1cm = W1r.transpose(0, 2, 1).reshape(IN_F, HID)
    W1P = np.concatenate([W1cm, As], axis=1).astype(np.float16)           # [128, 1056]
    WDP = Ad.astype(np.float16)                                           # [128, 32]
    W2cm = W2.reshape(H1, C1, OUT_EMB).transpose(1, 0, 2).reshape(HID, OUT_EMB)
    W2e = np.concatenate([W2cm, W2cm @ att_src2.T, W2cm @ att_dst2.T], axis=1)  # [1024, 66]
    W2P = np.ascontiguousarray(
        W2e.reshape(8, 128, 66).transpose(1, 0, 2).reshape(128, 8 * 66)
    ).astype(np.float16)
    C2T = np.tile(W2e.sum(axis=0)[None, :].astype(np.float32), (128, 1))  # [128, 66]

    d_ar = np.arange(128, dtype=np.float32)

    # per-core distinct sources (phase A computes only rows each core gathers)
    core_locs = []
    for c in range(NC):
        core_locs.append(np.unique(plan['src_tiles'][c]))
    MT = (max(len(l) for l in core_locs) + 127) // 128
    NLOC = MT * 128

    in_maps = []
    for c in range(NC):
        no = plan['node_order'][c]
        safe = np.where(no >= 0, no, 0)
        xtp = np.ascontiguousarray(x[safe].T).astype(np.float16)          # [128, NPAD]
        locs = core_locs[c]
        renum = np.zeros(N_NODES, np.int64)
        renum[locs] = np.arange(len(locs))
        xg = np.zeros((128, NLOC), np.float16)
        xg[:, 0:len(locs)] = x[locs].T.astype(np.float16)
        idx1 = wrap_idx(renum[plan['src_tiles'][c]], NB)
        idx2 = wrap_idx(plan['node_to_row'][plan['src_tiles'][c]], NB)
        seg = plan['seg_tiles'][c]                                        # [T, 128]
        onehot = (seg[:, :, None] == d_ar[None, None, :])                 # [T, e, d]
        s01t = np.ascontiguousarray(
            onehot.transpose(1, 0, 2).reshape(128, T * 128)).astype(np.float16)
        s10t = np.ascontiguousarray(
            onehot.transpose(2, 0, 1).reshape(128, T * 128)).astype(np.float16)
        im = {
            "XT_G": xg, "XTP": xtp,
            "W1P": W1P, "WDP": WDP, "W2P": W2P, "C2T": C2T,
            "IDX1": idx1, "IDX2": idx2,
            "S01T": s01t, "S10T": s10t,
        }
        if np.any(b1):
            b1cm = b1.reshape(H1, C1).T.reshape(HID)
            im["B1"] = np.tile(b1cm.reshape(1, HID).astype(np.float32), (128, 1))
        if np.any(b2):
            im["B2"] = np.tile(b2.reshape(1, OUT_EMB).astype(np.float32), (128, 1))
        in_maps.append(im)

    ncb = _get_program(NB, MT, bool(np.any(b1)), bool(np.any(b2)))

    from concourse.bass_utils import run_bass_kernel_spmd
    res = run_bass_kernel_spmd(
        ncb, in_maps, core_ids=list(range(NC)),
        trace=bool(int(os.environ.get("GAT_TRACE", "0"))),
    )

    out_full = np.zeros((N_NODES, OUT_EMB), np.float32)
    for c in range(NC):
        no = plan['node_order'][c]
        valid = no >= 0
        out_full[no[valid]] = res.results[c]["OUT"][valid]
    if _return_results:
        return out_full, res
    return out_full


# revision 53
# speedup vs baseline: 1.4509x; 1.1098x over previous
"""Two-layer GAT on 8 Trainium2 NeuronCores (Bass/Tile).

Strategy (graph/data parallel, dst-sharded):
- Host: add self-loops, sort edges by dst, shard dst-node ranges across 8
  cores, greedily pack each core's edges into 128-edge tiles grouped into
  128-node blocks (8 tiles/block); structure tables (wrapped gather idx,
  one-hot S01 [e,d] / S10 [d,e] per tile) precomputed on host.
- Device phase A (redundant on every core): HT[n] = [h1(n) | e_src1(n)]
  (fp16) for all 20000 nodes via x @ [W1|As] (TensorE), written to HBM.
  e_dst1 per packed node kept in SBUF (A2, interleaved). Layer-2 gather
  descriptors for the first blocks are pre-generated on idle GpSimd
  (SWDGE prepare_only on queues 1-3), triggered after the AllGather.
- Layer-1 edge phase, per 128-node block: one 1024-row SWDGE gather of
  HT[src]; e_dst broadcast dst->edges via S10 matmuls into PSUM;
  e = lrelu(es+ed); ex = exp(e - ln256) (shift cancels in softmax);
  messages formed in place (g1 *= ex per head); scatter-sum + ex-sum via
  S01^T matmuls into block PSUM; rinv = 1/sum ex; y = relu(o1)+exp(min(o1,0))
  (elu + 1, fp16); y^T via DMA-xbar transposes; layer-2 linear
  h2e = y^T @ W2e - colsum(W2e) (folds the elu "-1") inline; h2/es2 to
  H2TL, ed2 kept in SBUF.
- Halo exchange: AllGather of H2TL (h2 | es2 table) across 8 cores.
- Layer-2 edge phase: same machinery with H=1, C=64; gathers for blocks
  0..8 fire from pre-generated descriptors, the rest generate inline.
- Host: inverse-permute the 8 output shards into the full [20000, 64].
"""
import os
import sys
import numpy as np

sys.path.insert(0, '/opt/trn_rl_repo')

import concourse.bacc as bacc
import concourse.bass as bass
import concourse.mybir as mybir
import concourse.tile as tile
from concourse.masks import make_identity

F16 = mybir.dt.float16
F32 = mybir.dt.float32
I16 = mybir.dt.int16

N_NODES = 20000
IN_F = 128
HID = 1024          # 32 heads x 32 ch
H1, C1 = 32, 32
OUT_EMB = 64
NC = 8
SHARD = N_NODES // NC
K_TILES = 8
TILE_E = 128
NEG_SLOPE = 0.2
LN_SHIFT = float(np.log(256.0))
HT_ROW = 1152       # 1024 h + 32 es + 96 pad (2304B = 9*256)
EPS1 = float(1e-16 / 256.0)
PREP_DEPTH = 9      # layer-2 gathers pre-generated (3 SWDGE queues x 3)
AGB = 6             # layer-1 blocks per AllGather chunk (overlaps halo w/ L1)

# ---------------------------------------------------------------- host planning


def build_plan(edge_index: np.ndarray):
    ei = np.asarray(edge_index)
    loops = np.arange(N_NODES, dtype=ei.dtype)
    src = np.concatenate([ei[0], loops])
    dst = np.concatenate([ei[1], loops])
    order = np.argsort(dst, kind='stable')
    src_s = src[order].astype(np.int64)
    dst_s = dst[order].astype(np.int64)

    per_core = []
    max_nb = 0
    for c in range(NC):
        lo, hi = c * SHARD, (c + 1) * SHARD
        m = (dst_s >= lo) & (dst_s < hi)
        csrc, cdst = src_s[m], dst_s[m]
        nodes, starts, counts = np.unique(cdst, return_index=True, return_counts=True)
        blocks = []
        bi_nodes, bi_tiles = [], []
        t_src, t_seg = [], []

        def close_tile():
            nonlocal t_src, t_seg
            if t_src:
                bi_tiles.append((t_src, t_seg))
                t_src, t_seg = [], []

        def close_block():
            nonlocal bi_nodes, bi_tiles
            close_tile()
            if bi_nodes:
                blocks.append((bi_nodes, bi_tiles))
                bi_nodes, bi_tiles = [], []

        for n, st, cnt in zip(nodes, starts, counts):
            if len(t_src) + cnt > TILE_E:
                close_tile()
            need_new_tile = not t_src
            if len(bi_nodes) >= 128 or (need_new_tile and len(bi_tiles) >= K_TILES):
                close_block()
            local = len(bi_nodes)
            bi_nodes.append(int(n))
            t_src.extend(csrc[st:st + cnt].tolist())
            t_seg.extend([local] * int(cnt))
        close_block()
        per_core.append(blocks)
        max_nb = max(max_nb, len(blocks))

    NB = max_nb
    T = NB * K_TILES
    NPAD = NB * 128
    plan = {
        'NB': NB, 'T': T, 'NPAD': NPAD,
        'src_tiles': np.zeros((NC, T, TILE_E), np.int64),
        'seg_tiles': np.full((NC, T, TILE_E), -1.0, np.float32),
        'node_order': np.full((NC, NPAD), -1, np.int64),
    }
    for c, blocks in enumerate(per_core):
        for b, (bnodes, btiles) in enumerate(blocks):
            for r, n in enumerate(bnodes):
                plan['node_order'][c, b * 128 + r] = n
            for k, (tsrc, tseg) in enumerate(btiles):
                t = b * K_TILES + k
                plan['src_tiles'][c, t, :len(tsrc)] = tsrc
                plan['seg_tiles'][c, t, :len(tseg)] = tseg
    # h2 table rows follow the chunked-AllGather layout:
    # chunk g (AGB blocks) holds cores' sections back to back
    AGR = AGB * 128
    node_to_row = np.zeros(N_NODES, np.int64)
    for c in range(NC):
        valid = plan['node_order'][c] >= 0
        r = np.nonzero(valid)[0]
        g = r // AGR
        rows_g = np.minimum(AGR, NPAD - g * AGR)
        row = NC * AGR * g + c * rows_g + (r - g * AGR)
        node_to_row[plan['node_order'][c][valid]] = row
    plan['node_to_row'] = node_to_row
    return plan


def wrap_idx(idx_tiles: np.ndarray, NB: int) -> np.ndarray:
    """[T,128] -> dma_gather wrapped layout [128, NB*64] int16 (batch = 8 tiles)."""
    out = np.zeros((128, NB * 64), np.int16)
    flat = idx_tiles.reshape(NB, K_TILES * TILE_E)
    for b in range(NB):
        w = np.zeros((16, 64), np.int16)
        v = flat[b]
        idx = np.arange(1024)
        w[idx % 16, idx // 16] = v.astype(np.int16)
        out[:, b * 64:(b + 1) * 64] = np.tile(w, (8, 1))
    return out


# ---------------------------------------------------------------- device program

def build_program(NB, MT, b1_nonzero, b2_nonzero, use_collective=True):
    T = NB * K_TILES
    NPAD = NB * 128
    NLOC = MT * 128     # per-core distinct-source table rows (padded)
    use_mq = bool(int(os.environ.get("GAT_MQ", "1")))
    # prepare_only descriptor pre-generation deadlocks on hardware (works in
    # the scheduler sim); keep the path behind a flag, default off
    use_prep = use_mq and bool(int(os.environ.get("GAT_PREP", "0")))
    use_dmatp = bool(int(os.environ.get("GAT_DMATP", "1")))
    n_prep = min(PREP_DEPTH, NB) if use_prep else 0

    nc = bacc.Bacc("TRN2", target_bir_lowering=False, debug=False, num_devices=NC,
                   num_swdge_queues=4 if use_mq else 1,
                   dynamic_dma_scratch_size=49152 if use_prep else 32768)

    def din(name, shape, dt):
        return nc.dram_tensor(name, shape, dt, kind="ExternalInput")

    XT_G = din("XT_G", [128, NLOC], F16)
    XTP = din("XTP", [128, NPAD], F16)
    W1P = din("W1P", [128, 1056], F16)
    WDP = din("WDP", [128, 32], F16)
    W2P = din("W2P", [128, 8 * 66], F16)
    C2T = din("C2T", [128, 66], F32)
    IDX1 = din("IDX1", [128, NB * 64], I16)
    IDX2 = din("IDX2", [128, NB * 64], I16)
    S01T = din("S01T", [128, T * 128], F16)
    S10T = din("S10T", [128, T * 128], F16)
    if b1_nonzero:
        B1 = din("B1", [128, HID], F32)
    if b2_nonzero:
        B2 = din("B2", [128, OUT_EMB], F32)

    OUT = nc.dram_tensor("OUT", [NPAD, OUT_EMB], F32, kind="ExternalOutput")

    HT = nc.dram_tensor("HT", [NLOC, HT_ROW], F16)
    H2TL = nc.dram_tensor("H2TL", [NPAD, 128], F16)
    H2TF = nc.dram_tensor("H2TF", [NC * NPAD, 128], F16, addr_space="Shared")

    with tile.TileContext(nc) as tc:
        with (
            tc.tile_pool(name="const", bufs=1) as cpool,
            tc.tile_pool(name="sb", bufs=2) as sb,
            tc.tile_pool(name="psum", bufs=2, space="PSUM") as pp,
        ):
            # ---- resident tables
            xtp = cpool.tile([128, NPAD], F16, tag="xtp")
            nc.sync.dma_start(xtp[:], XTP[:, :])
            w1p = cpool.tile([128, 1056], F16, tag="w1p")
            nc.sync.dma_start(w1p[:], W1P[:, :])
            wdp = cpool.tile([128, 32], F16, tag="wdp")
            nc.sync.dma_start(wdp[:], WDP[:, :])
            w2p = cpool.tile([128, 8 * 66], F16, tag="w2p")
            nc.sync.dma_start(w2p[:], W2P[:, :])
            c2t = cpool.tile([128, 66], F32, tag="c2t")
            nc.sync.dma_start(c2t[:], C2T[:, :])
            idx1 = cpool.tile([128, NB * 64], I16, tag="idx1")
            nc.sync.dma_start(idx1[:], IDX1[:, :])
            idx2 = cpool.tile([128, NB * 64], I16, tag="idx2")
            nc.sync.dma_start(idx2[:], IDX2[:, :])
            if b1_nonzero:
                b1t = cpool.tile([128, HID], F32, tag="b1t")
                nc.sync.dma_start(b1t[:], B1[:, :])
            if b2_nonzero:
                b2t = cpool.tile([128, OUT_EMB], F32, tag="b2t")
                nc.sync.dma_start(b2t[:], B2[:, :])
            ed1 = cpool.tile([128, NB * 32], F16, tag="ed1")
            ed2sb = cpool.tile([128, NB], F16, tag="ed2sb")
            negshift = cpool.tile([128, 1], F32, tag="negshift")
            nc.gpsimd.memset(negshift[:], -LN_SHIFT)
            ident = cpool.tile([128, 128], F16, tag="ident")
            make_identity(nc, ident[:])

            g2sems = [nc.alloc_semaphore(f"g2dma{q}") for q in range(3)]
            g2_tiles = {}

            g2_bufs = PREP_DEPTH + 3 if use_prep else 6

            def emit_g2_gather(j, prepare):
                g2t = sb.tile([128, K_TILES, 128], F16, tag="g2",
                              bufs=g2_bufs, name=f"g2_{j}")
                g2_tiles[j] = g2t
                if prepare:
                    nc.gpsimd.dma_gather(
                        out_ap=g2t[:], in_ap=H2TF[:, :],
                        idxs_ap=idx2[:, j * 64:(j + 1) * 64],
                        num_idxs=1024, num_idxs_reg=1024, elem_size=128,
                        queue_num=1 + j % 3, prepare_only=True, sem=g2sems[j % 3])
                    return
                for hf in range(2):
                    nc.gpsimd.dma_gather(
                        out_ap=g2t[:, hf * 4:(hf + 1) * 4, :], in_ap=H2TF[:, :],
                        idxs_ap=idx2[:, j * 64 + hf * 32:j * 64 + (hf + 1) * 32],
                        num_idxs=512, num_idxs_reg=512, elem_size=128,
                        queue_num=(2 * j + hf) % 4 if use_mq else 0)

            # ================= phase A: HT = [h | es] for this core's
            # distinct sources only (host-renumbered); layer-2 gather preps
            # + per-block e_dst1 (A2) interleaved
            n_a_tiles = MT
            CHUNK = 20 * 128
            prep_spacing = max(1, (n_a_tiles - 20) // max(n_prep, 1))
            prep_at = {20 + prep_spacing * j: j for j in range(n_prep)}
            a2_every = max(1, n_a_tiles // NB)
            xa = None
            for m in range(n_a_tiles):
                n0 = m * 128
                nn = 128
                if m % 20 == 0:
                    c0 = m * 128
                    cw = min(CHUNK, NLOC - c0)
                    xa = sb.tile([128, CHUNK], F16, tag="xa")
                    nc.sync.dma_start(xa[:, 0:cw], XT_G[:, c0:c0 + cw])
                # three independent psum streams (tp/blk banks idle in phase A)
                pa0 = pp.tile([128, 512], F32, tag="tp")
                pa1 = pp.tile([128, 512], F32, tag="blk")
                paes = pp.tile([128, 1024], F32, tag="ob")
                lhsT = xa[:, n0 - (m // 20) * CHUNK:n0 - (m // 20) * CHUNK + nn]
                stage = sb.tile([128, HT_ROW], F16, tag="aStage", bufs=3)
                nc.tensor.matmul(pa0[0:nn, :], lhsT, w1p[:, 0:512], start=True, stop=True)
                nc.scalar.copy(stage[0:nn, 0:512], pa0[0:nn, :])
                nc.tensor.matmul(pa1[0:nn, :], lhsT, w1p[:, 512:1024], start=True, stop=True)
                nc.tensor.matmul(paes[0:nn, 0:32], lhsT, w1p[:, 1024:1056], start=True, stop=True)
                nc.vector.tensor_copy(stage[0:nn, 512:1024], pa1[0:nn, :])
                nc.vector.tensor_copy(stage[0:nn, 1024:1056], paes[0:nn, 0:32])
                nc.sync.dma_start(HT[n0:n0 + nn, :], stage[0:nn, :])
                # A2: e_dst1 per packed block, kept in SBUF
                if m % a2_every == 0 and m // a2_every < NB:
                    b = m // a2_every
                    ped = pp.tile([128, 1024], F32, tag="ob")
                    nc.tensor.matmul(ped[:, 0:32], xtp[:, b * 128:(b + 1) * 128], wdp[:],
                                     start=True, stop=True)
                    nc.vector.tensor_copy(ed1[:, b * 32:(b + 1) * 32], ped[:, 0:32])
                if m in prep_at:
                    emit_g2_gather(prep_at[m], prepare=True)

            # ================= layer-1 edge phase (+ inline layer-2 linear) ==
            n_ag = (NB + AGB - 1) // AGB

            def emit_ag(g):
                r0 = g * AGB * 128
                rows = min(AGB * 128, NPAD - r0)
                base = NC * AGB * 128 * g
                if use_collective:
                    nc.gpsimd.collective_compute(
                        "AllGather",
                        mybir.AluOpType.bypass,
                        ins=[H2TL[r0:r0 + rows, :].opt()],
                        outs=[H2TF[base:base + NC * rows, :].opt()],
                        replica_groups=[list(range(NC))],
                    )
                else:
                    nc.sync.dma_start(H2TF[base:base + rows, :],
                                      H2TL[r0:r0 + rows, :])

            for b in range(NB):
                # two 512-row gathers: a full-window 1024-row gather serializes
                # desc-gen behind its own drain on the SWDGE ring
                g1a = sb.tile([128, 4, HT_ROW], F16, tag="g1", name="g1a", bufs=6)
                nc.gpsimd.dma_gather(
                    out_ap=g1a[:], in_ap=HT[:, :],
                    idxs_ap=idx1[:, b * 64:b * 64 + 32],
                    num_idxs=512, num_idxs_reg=512, elem_size=HT_ROW,
                    queue_num=(2 * b) % 4 if (use_mq and not use_prep) else 0)
                g1b = sb.tile([128, 4, HT_ROW], F16, tag="g1", name="g1b", bufs=6)
                nc.gpsimd.dma_gather(
                    out_ap=g1b[:], in_ap=HT[:, :],
                    idxs_ap=idx1[:, b * 64 + 32:(b + 1) * 64],
                    num_idxs=512, num_idxs_reg=512, elem_size=HT_ROW,
                    queue_num=(2 * b + 1) % 4 if (use_mq and not use_prep) else 0)
                g1h = [g1a, g1b]
                s01 = sb.tile([128, K_TILES, 128], F16, tag="s01")
                nc.sync.dma_start(s01[:], S01T[:, b * 1024:(b + 1) * 1024])
                s10 = sb.tile([128, K_TILES, 128], F16, tag="s10")
                nc.sync.dma_start(s10[:], S10T[:, b * 1024:(b + 1) * 1024])

                # block psum: [0:256] ed_e | [256:288] sbp | [288:354] ph
                blk = pp.tile([128, 512], F32, tag="blk")
                ed_e = blk[:, 0:256]
                for k in range(K_TILES):
                    nc.tensor.matmul(ed_e[:, k * 32:(k + 1) * 32], s10[:, k, :],
                                     ed1[:, b * 32:(b + 1) * 32],
                                     start=True, stop=True)
                # e chain (fp16)
                e_f = sb.tile([128, K_TILES * 32], F16, tag="e_f")
                for hf in range(2):
                    nc.vector.tensor_tensor(
                        out=e_f[:, hf * 128:(hf + 1) * 128]
                             .rearrange("p (k h) -> p k h", k=4),
                        in0=g1h[hf][:, :, 1024:1056],
                        in1=ed_e[:, hf * 128:(hf + 1) * 128]
                             .rearrange("p (k h) -> p k h", k=4),
                        op=mybir.AluOpType.add)
                lr = sb.tile([128, K_TILES * 32], F16, tag="lr")
                nc.vector.scalar_tensor_tensor(
                    out=lr[:], in0=e_f[:], scalar=NEG_SLOPE, in1=e_f[:],
                    op0=mybir.AluOpType.mult, op1=mybir.AluOpType.max)
                ex = sb.tile([128, K_TILES * 32], F16, tag="ex")
                nc.scalar.activation(ex[:], lr[:], mybir.ActivationFunctionType.Exp,
                                     bias=negshift[:, 0:1])
                # messages (channel-major h: inner dim head, stride-1 on both
                # DVE operands); two halves so scatter matmuls overlap
                ob = pp.tile([128, HID], F32, tag="ob")
                sbp = blk[:, 256:288]
                for hf in range(2):
                    mh = sb.tile([128, 4, HID], F16, tag="msg", name=f"msg{hf}", bufs=4)
                    nc.vector.tensor_tensor(
                        out=mh[:].rearrange("p k (c h) -> p k c h", c=C1),
                        in0=g1h[hf][:, :, 0:1024].rearrange("p k (c h) -> p k c h", c=C1),
                        in1=ex[:, hf * 128:(hf + 1) * 128]
                             .rearrange("p (k h) -> p k h", k=4)
                             .unsqueeze(2).to_broadcast([128, 4, C1, H1]),
                        op=mybir.AluOpType.mult)
                    for k in range(hf * 4, hf * 4 + 4):
                        lhsT = s01[:, k, :]
                        nc.tensor.matmul(sbp[:, 0:32], lhsT, ex[:, k * 32:(k + 1) * 32],
                                         start=(k == 0), stop=(k == K_TILES - 1))
                        nc.tensor.matmul(ob[:, 0:512], lhsT, mh[:, k % 4, 0:512],
                                         start=(k == 0), stop=(k == K_TILES - 1))
                        nc.tensor.matmul(ob[:, 512:1024], lhsT, mh[:, k % 4, 512:1024],
                                         start=(k == 0), stop=(k == K_TILES - 1))

                # ---- block finishing (fp16 elu path; y = elu + 1)
                sp = sb.tile([128, 32], F32, tag="sp")
                nc.vector.tensor_scalar_add(sp[:], sbp[:, 0:32], EPS1)
                rinv = sb.tile([128, 32], F32, tag="rinv")
                nc.vector.reciprocal(rinv[:], sp[:])
                o1h = sb.tile([128, HID], F16, tag="o1h")
                nc.vector.tensor_tensor(
                    out=o1h[:].rearrange("p (c h) -> p c h", c=C1),
                    in0=ob[:].rearrange("p (c h) -> p c h", c=C1),
                    in1=rinv[:].unsqueeze(1).to_broadcast([128, C1, H1]),
                    op=mybir.AluOpType.mult)
                if b1_nonzero:
                    nc.vector.tensor_tensor(out=o1h[:], in0=o1h[:], in1=b1t[:, :],
                                            op=mybir.AluOpType.add)
                # y = elu(x)+1 = relu(x) + min(exp(x), 1): exact (exp overflow
                # saturates to inf, min clamps); single DVE pass after two
                # scalar-engine activations
                texp = sb.tile([128, HID], F16, tag="texp")
                nc.scalar.activation(texp[:], o1h[:], mybir.ActivationFunctionType.Exp)
                trel = sb.tile([128, HID], F16, tag="trel")
                nc.scalar.activation(trel[:], o1h[:], mybir.ActivationFunctionType.Relu)
                yb = sb.tile([128, HID], F16, tag="yb")
                nc.vector.scalar_tensor_tensor(
                    out=yb[:], in0=texp[:], scalar=1.0, in1=trel[:],
                    op0=mybir.AluOpType.min, op1=mybir.AluOpType.add)
                # y^T via PE transposes into one f16 psum bank
                tpb = pp.tile([128, HID], F16, tag="tp", bufs=2)
                for kk in range(8):
                    nc.tensor.transpose(tpb[:, kk * 128:(kk + 1) * 128],
                                        yb[:, kk * 128:(kk + 1) * 128], ident[:])
                tstage = sb.tile([128, HID], F16, tag="tst")
                nc.scalar.copy(tstage[:], tpb[:])
                # layer-2 linear: h2e = y^T @ W2e - colsum(W2e)  (elu -1 folded)
                ph = blk[:, 288:354]
                for k in range(8):
                    nc.tensor.matmul(ph[:, 0:66], tstage[:, k * 128:(k + 1) * 128],
                                     w2p[:, k * 66:(k + 1) * 66],
                                     start=(k == 0), stop=(k == 7))
                h2s = sb.tile([128, 128], F16, tag="h2s")
                nc.vector.tensor_tensor(out=h2s[:, 0:65], in0=ph[:, 0:65],
                                        in1=c2t[:, 0:65], op=mybir.AluOpType.subtract)
                nc.vector.tensor_tensor(out=ed2sb[:, b:b + 1], in0=ph[:, 65:66],
                                        in1=c2t[:, 65:66], op=mybir.AluOpType.subtract)
                nc.sync.dma_start(H2TL[b * 128:(b + 1) * 128, :], h2s[:])
                # halo exchange, chunked so it overlaps the rest of layer 1
                if (b + 1) % AGB == 0:
                    emit_ag(b // AGB)
            for g in range(NB // AGB, n_ag):
                emit_ag(g)

            # ================= layer-2 edge phase =================
            for j in range(n_prep):
                nc.gpsimd.trigger_dma(count=1, queue_num=1 + j % 3)
            for b in range(NB):
                if b + n_prep < NB:
                    emit_g2_gather(b + n_prep, prepare=False)
                g2 = g2_tiles[b]
                s01b = sb.tile([128, K_TILES, 128], F16, tag="s01L2", bufs=3)
                nc.sync.dma_start(s01b[:], S01T[:, b * 1024:(b + 1) * 1024])
                s10b = sb.tile([128, K_TILES, 128], F16, tag="s10L2", bufs=3)
                nc.sync.dma_start(s10b[:], S10T[:, b * 1024:(b + 1) * 1024])

                blk2 = pp.tile([128, 512], F32, tag="blk")
                ed2_e = blk2[:, 0:256]
                for k in range(K_TILES):
                    nc.tensor.matmul(ed2_e[:, k:k + 1], s10b[:, k, :],
                                     ed2sb[:, b:b + 1], start=True, stop=True)
                e2 = sb.tile([128, K_TILES], F16, tag="e2", bufs=3)
                nc.vector.tensor_tensor(
                    out=e2[:].unsqueeze(2),
                    in0=g2[:, :, 64:65],
                    in1=ed2_e[:, 0:K_TILES].unsqueeze(2),
                    op=mybir.AluOpType.add)
                nc.vector.scalar_tensor_tensor(
                    out=e2[:], in0=e2[:], scalar=NEG_SLOPE, in1=e2[:],
                    op0=mybir.AluOpType.mult, op1=mybir.AluOpType.max)
                ex2 = sb.tile([128, K_TILES], F16, tag="ex2", bufs=3)
                nc.scalar.activation(ex2[:], e2[:], mybir.ActivationFunctionType.Exp,
                                     bias=negshift[:, 0:1])
                msg2 = sb.tile([128, K_TILES, OUT_EMB], F16, tag="msg2", bufs=3)
                nc.vector.tensor_tensor(
                    out=msg2[:],
                    in0=g2[:, :, 0:64],
                    in1=ex2[:].unsqueeze(2).to_broadcast([128, K_TILES, OUT_EMB]),
                    op=mybir.AluOpType.mult)
                ob2 = pp.tile([128, HID], F32, tag="ob")
                sb2 = blk2[:, 256:288]
                for k in range(K_TILES):
                    lhsT = s01b[:, k, :]
                    nc.tensor.matmul(sb2[:, 0:1], lhsT, ex2[:, k:k + 1],
                                     start=(k == 0), stop=(k == K_TILES - 1))
                    nc.tensor.matmul(ob2[:, 0:64], lhsT, msg2[:, k, :],
                                     start=(k == 0), stop=(k == K_TILES - 1))
                sp2 = sb.tile([128, 1], F32, tag="sp2", bufs=3)
                nc.vector.tensor_scalar_add(sp2[:], sb2[:, 0:1], EPS1)
                rinv2 = sb.tile([128, 1], F32, tag="rinv2", bufs=3)
                nc.vector.reciprocal(rinv2[:], sp2[:])
                o2 = sb.tile([128, OUT_EMB], F32, tag="o2", bufs=3)
                nc.vector.tensor_scalar_mul(o2[:], ob2[:, 0:64], rinv2[:, 0:1])
                if b2_nonzero:
                    nc.vector.tensor_tensor(out=o2[:], in0=o2[:], in1=b2t[:, :],
                                            op=mybir.AluOpType.add)
                nc.sync.dma_start(OUT[b * 128:(b + 1) * 128, :], o2[:])

    nc.compile()
    return nc


# ---------------------------------------------------------------- driver

_CACHE = {}


def _get_program(NB, MT, b1_nonzero, b2_nonzero):
    key = (NB, MT, b1_nonzero, b2_nonzero)
    if key not in _CACHE:
        _CACHE[key] = build_program(NB, MT, b1_nonzero, b2_nonzero)
    return _CACHE[key]


def kernel(x, edge_index, W1, att_src1, att_dst1, b1, W2, att_src2, att_dst2, b2,
           _return_results=False):
    x = np.asarray(x); edge_index = np.asarray(edge_index)
    W1 = np.asarray(W1); att_src1 = np.asarray(att_src1); att_dst1 = np.asarray(att_dst1)
    b1 = np.asarray(b1); W2 = np.asarray(W2)
    att_src2 = np.asarray(att_src2); att_dst2 = np.asarray(att_dst2); b2 = np.asarray(b2)

    plan = build_plan(edge_index)
    NB, T, NPAD = plan['NB'], plan['T'], plan['NPAD']

    # fused weights (host, fp32 math then fp16); hidden in channel-major
    # layout (col c*32+h) so the per-head DVE broadcasts keep stride-1 inner
    W1r = W1.reshape(IN_F, H1, C1)
    As = np.einsum('fhc,hc->fh', W1r, att_src1)
    Ad = np.einsum('fhc,hc->fh', W1r, att_dst1)
    W1cm = W1r.transpose(0, 2, 1).reshape(IN_F, HID)
    W1P = np.concatenate([W1cm, As], axis=1).astype(np.float16)           # [128, 1056]
    WDP = Ad.astype(np.float16)                                           # [128, 32]
    W2cm = W2.reshape(H1, C1, OUT_EMB).transpose(1, 0, 2).reshape(HID, OUT_EMB)
    W2e = np.concatenate([W2cm, W2cm @ att_src2.T, W2cm @ att_dst2.T], axis=1)  # [1024, 66]
    W2P = np.ascontiguousarray(
        W2e.reshape(8, 128, 66).transpose(1, 0, 2).reshape(128, 8 * 66)
    ).astype(np.float16)
    C2T = np.tile(W2e.sum(axis=0)[None, :].astype(np.float32), (128, 1))  # [128, 66]

    d_ar = np.arange(128, dtype=np.float32)

    # per-core distinct sources (phase A computes only rows each core gathers)
    core_locs = []
    for c in range(NC):
        core_locs.append(np.unique(plan['src_tiles'][c]))
    MT = (max(len(l) for l in core_locs) + 127) // 128
    NLOC = MT * 128

    in_maps = []
    for c in range(NC):
        no = plan['node_order'][c]
        safe = np.where(no >= 0, no, 0)
        xtp = np.ascontiguousarray(x[safe].T).astype(np.float16)          # [128, NPAD]
        locs = core_locs[c]
        renum = np.zeros(N_NODES, np.int64)
        renum[locs] = np.arange(len(locs))
        xg = np.zeros((128, NLOC), np.float16)
        xg[:, 0:len(locs)] = x[locs].T.astype(np.float16)
        idx1 = wrap_idx(renum[plan['src_tiles'][c]], NB)
        idx2 = wrap_idx(plan['node_to_row'][plan['src_tiles'][c]], NB)
        seg = plan['seg_tiles'][c]                                        # [T, 128]
        onehot = (seg[:, :, None] == d_ar[None, None, :])                 # [T, e, d]
        s01t = np.ascontiguousarray(
            onehot.transpose(1, 0, 2).reshape(128, T * 128)).astype(np.float16)
        s10t = np.ascontiguousarray(
            onehot.transpose(2, 0, 1).reshape(128, T * 128)).astype(np.float16)
        im = {
            "XT_G": xg, "XTP": xtp,
            "W1P": W1P, "WDP": WDP, "W2P": W2P, "C2T": C2T,
            "IDX1": idx1, "IDX2": idx2,
            "S01T": s01t, "S10T": s10t,
        }
        if np.any(b1):
            b1cm = b1.reshape(H1, C1).T.reshape(HID)
            im["B1"] = np.tile(b1cm.reshape(1, HID).astype(np.float32), (128, 1))
        if np.any(b2):
            im["B2"] = np.tile(b2.reshape(1, OUT_EMB).astype(np.float32), (128, 1))
        in_maps.append(im)

    ncb = _get_program(NB, MT, bool(np.any(b1)), bool(np.any(b2)))

    from concourse.bass_utils import run_bass_kernel_spmd
    res = run_bass_kernel_spmd(
        ncb, in_maps, core_ids=list(range(NC)),
        trace=bool(int(os.environ.get("GAT_TRACE", "0"))),
    )

    out_full = np.zeros((N_NODES, OUT_EMB), np.float32)
    for c in range(NC):
        no = plan['node_order'][c]
        valid = no >= 0
        out_full[no[valid]] = res.results[c]["OUT"][valid]
    if _return_results:
        return out_full, res
    return out_full
